# revision 25
# baseline (speedup 1.0000x reference)
"""HLMPNN (hierarchical layered MPNN) Bass kernel for 8 TRN2 NeuronCores — v2.

Strategy (graph/data parallel):
  - Nodes assigned to 392 degree-balanced bins of 128 (greedy packing on
    in-degree), 49 bins (groups) per core -> per-(core,group) slot counts
    nearly equal, minimizing block padding and max-over-core waste.
  - Edge MLP decomposed: per-node Q = relu(z@W1+b1) bf16, AllGather Q,
    per-edge dma_gather of Q rows (int16 idx, two half-tables), scatter-add
    via one-hot matmuls into PSUM, then mean-normalize and apply W2.
  - Self-loops are not gathered: added via an identity matmul from local Q.
  - dma_gather calls round-robin across 4 SWDGE queues (Q7 core pairs) for
    parallel descriptor generation (measured ~2.8x).
  - All matmuls bf16 (fp32 is 4 cyc/row on the PE); z-state in bf16;
    LayerNorm statistics fp32.
  - beta-weighted output accumulated in SBUF fp32; one DRAM write at end.
"""
import numpy as np
import ml_dtypes

import concourse.bass as bass
import concourse.bass2jax as _b2j
import concourse.mybir as mybir

_orig_hook = _b2j.neuronx_cc_hook
def _dbg_hook(*a, **k):
    try:
        return _orig_hook(*a, **k)
    except BaseException:
        import traceback
        traceback.print_exc()
        raise
_b2j.neuronx_cc_hook = _dbg_hook
import concourse.tile as tile
from concourse import bacc
from concourse.bass_utils import run_bass_kernel_spmd
from concourse.masks import make_identity

F32 = mybir.dt.float32
BF16 = mybir.dt.bfloat16
I16 = mybir.dt.int16
AF = mybir.ActivationFunctionType
OP = mybir.AluOpType

CORES = 8
N = 50000
IN_CH = 128
HID = 256
MSG = 128
L = 10
EPS = 1e-5
G = 49                       # groups (bins) per core
NPAD = G * 128               # 6272 node slots per core
NTOT = NPAD * CORES          # 50176
HALF = NTOT // 2             # 25088
PADDST = 200.0
NQ = 4                       # SWDGE queues
BATCH = 4                    # groups per MLP batch

BF = ml_dtypes.bfloat16


def _preprocess(edge_index):
    src = np.asarray(edge_index[0], np.int64)
    dst = np.asarray(edge_index[1], np.int64)

    deg = np.bincount(dst, minlength=N).astype(np.int64)   # in-degree, no loop
    counts_node = (deg + 1).astype(np.float64)             # with self-loop

    # --- degree-balanced assignment of nodes to 392 bins of <=128 ---
    import heapq
    NB = CORES * G
    order = np.argsort(-deg, kind="stable")
    heap = [(0, b) for b in range(NB)]
    heapq.heapify(heap)
    bin_of = np.empty(N, np.int32)
    pos_of = np.empty(N, np.int32)
    bin_cnt = np.zeros(NB, np.int32)
    for n in order:
        spill = []
        while True:
            load, b = heapq.heappop(heap)
            if bin_cnt[b] < 128:
                break
            spill.append((load, b))
        bin_of[n] = b
        pos_of[n] = bin_cnt[b]
        bin_cnt[b] += 1
        heapq.heappush(heap, (load + int(deg[n]), b))
        # bins at capacity stay out of the heap permanently
    core_of = (bin_of // G).astype(np.int64)
    grp_of = (bin_of % G).astype(np.int64)
    loc_of = grp_of * 128 + pos_of                  # local row in [0, 6272)
    glob_of = core_of * NPAD + loc_of               # row in qfull

    # --- per-edge slot tables (dst-owner core gathers src rows) ---
    e_core = core_of[dst]
    e_grp = grp_of[dst]
    e_nloc = pos_of[dst].astype(np.int64)
    e_src_glob = glob_of[src]
    e_h = (e_src_glob >= HALF).astype(np.int64)
    e_idx = e_src_glob - e_h * HALF

    order_e = np.lexsort((e_idx, e_h, e_grp, e_core))
    so, sg, sh = e_core[order_e], e_grp[order_e], e_h[order_e]
    si, sn = e_idx[order_e], e_nloc[order_e]
    key = ((so * G) + sg) * 2 + sh
    bounds = np.searchsorted(key, np.arange(CORES * G * 2 + 1))

    cnt = (bounds[1:] - bounds[:-1]).reshape(CORES, G, 2)
    B = np.maximum(1, -(-cnt.max(axis=0) // 128))   # [G, 2] block counts
    seg_off = np.zeros((G, 2), np.int64)
    off = 0
    for g in range(G):
        for h in range(2):
            seg_off[g, h] = off
            off += int(B[g, h]) * 128
    totslots = off
    totb = totslots // 128

    idx16 = np.zeros((CORES, totslots), np.int16)   # pad idx 0 (a real row)
    dstv = np.full((CORES, totslots), PADDST, np.float32)
    for r in range(CORES):
        for g in range(G):
            for h in range(2):
                k = (r * G + g) * 2 + h
                lo, hi = bounds[k], bounds[k + 1]
                o = seg_off[g, h]
                idx16[r, o:o + hi - lo] = si[lo:hi].astype(np.int16)
                dstv[r, o:o + hi - lo] = sn[lo:hi].astype(np.float32)

    idx_pack = np.tile(
        idx16.reshape(CORES, totslots // 16, 16).transpose(0, 2, 1), (1, 8, 1)
    )  # [CORES, 128, totslots//16]
    dstv_cols = dstv.reshape(CORES, totb, 128).transpose(0, 2, 1)  # [C,128,totb]

    cinv = np.ones((CORES, NPAD), np.float64)
    cinv[core_of, loc_of] = 1.0 / counts_node

    perm = (core_of, loc_of)
    return B, seg_off, idx_pack, dstv_cols, cinv, totb, perm


def _build(B, seg_off, totb, betas, nlayers=L):
    nc = bacc.Bacc(None, target_bir_lowering=False, debug=False,
                   num_swdge_queues=NQ)

    xT_d = nc.dram_tensor("xT", [128, NPAD], BF16, kind="ExternalInput")
    win_d = nc.dram_tensor("win", [128, HID], BF16, kind="ExternalInput")
    binrow_d = nc.dram_tensor("binrow", [1, HID], BF16, kind="ExternalInput")
    w1_d = nc.dram_tensor("w1t", [nlayers, 128, 2, 128], BF16, kind="ExternalInput")
    w2_d = nc.dram_tensor("w2t", [nlayers, 128, 2, 128], BF16, kind="ExternalInput")
    u1_d = nc.dram_tensor("u1t", [nlayers, 128, 4, 128], BF16, kind="ExternalInput")
    u2_d = nc.dram_tensor("u2t", [nlayers, 128, 2, HID], BF16, kind="ExternalInput")
    rows_d = nc.dram_tensor("rows", [nlayers, 6 * HID], BF16, kind="ExternalInput")
    idx_d = nc.dram_tensor("idxp", [128, totb * 8], I16, kind="ExternalInput")
    dstv_d = nc.dram_tensor("dstv", [128, totb], BF16, kind="ExternalInput")
    cinv_d = nc.dram_tensor("cinv", [128, NPAD], BF16, kind="ExternalInput")
    out_d = nc.dram_tensor("out", [NPAD, HID], F32, kind="ExternalOutput")

    nbmax = int(B.max())
    qcall = [0]

    # group batches: [0..3], [4..7], ..., [48]
    batches = []
    g0 = 0
    while g0 < G:
        ng = min(BATCH, G - g0)
        batches.append((g0, ng))
        g0 += ng

    with tile.TileContext(nc) as tc:
        with (
            tc.tile_pool(name="persist", bufs=1) as pp,
            tc.tile_pool(name="wpool", bufs=2) as wp,
            tc.tile_pool(name="stream", bufs=3) as sp,
            tc.tile_pool(name="gath", bufs=8) as gp,
            tc.tile_pool(name="psC", bufs=2, space="PSUM") as psC,
            tc.tile_pool(name="psS", bufs=2, space="PSUM") as psS,
            tc.tile_pool(name="psB", bufs=1, space="PSUM") as psB,
            tc.tile_pool(name="dram", bufs=2, space="DRAM") as dp,
        ):
            # ---- persistent state ----
            zT = pp.tile([128, 2 * NPAD], BF16)      # z feature-major (h, node)
            acc = pp.tile([128, G, HID], F32)        # beta-weighted accumulator
            qkeep = pp.tile([128, G, MSG], BF16)     # own Q node-major per group
            cinv_sb = pp.tile([128, NPAD], BF16)
            dstv_sb = pp.tile([128, totb], BF16)
            idx_sb = pp.tile([128, totb * 8], I16)
            iota_bf = pp.tile([128, 128], BF16)
            ident = pp.tile([128, 128], F32)
            ident_bf = pp.tile([128, 128], BF16)
            ones_r = pp.tile([1, 512], BF16)
            win_sb = pp.tile([128, HID], BF16)
            binrow_sb = pp.tile([1, HID], BF16)

            nc.sync.dma_start(out=cinv_sb[:], in_=cinv_d[:])
            nc.sync.dma_start(out=dstv_sb[:], in_=dstv_d[:])
            nc.sync.dma_start(out=idx_sb[:], in_=idx_d[:])
            nc.sync.dma_start(out=win_sb[:], in_=win_d[:])
            nc.sync.dma_start(out=binrow_sb[:], in_=binrow_d[:])

            iota_i = sp.tile([128, 128], mybir.dt.int32, tag="ioi")
            nc.gpsimd.iota(iota_i[:], pattern=[[1, 128]], base=0, channel_multiplier=0)
            nc.vector.tensor_copy(out=iota_bf[:], in_=iota_i[:])
            make_identity(nc, ident[:])
            nc.vector.tensor_copy(out=ident_bf[:], in_=ident[:])
            nc.vector.memset(ones_r[:], 1.0)
            # warm the gather pool: -1-trimmed pad slots leave stale SBUF
            # content that the one-hot zeros multiply; ensure it is finite
            for _ in range(8):
                gw = gp.tile([128, nbmax, 128], BF16, tag="gat")
                nc.vector.memset(gw[:], 0.0)

            def zview():
                return zT[:].rearrange("p (h n) -> p h n", h=2)

            def zcols(h, g0, ng=1):
                return slice(h * NPAD + g0 * 128, h * NPAD + (g0 + ng) * 128)

            # ---- z0 = x @ Win + bin ----
            for g in range(G):
                xg = sp.tile([128, 128], BF16, tag="xg")
                nc.sync.dma_start(out=xg[:], in_=xT_d[:, g * 128:(g + 1) * 128])
                zq = psC.tile([128, HID], F32, tag="C", space="PSUM")
                nc.tensor.matmul(zq[:], lhsT=xg[:],
                                 rhs=win_sb[:], start=True, stop=False)
                nc.tensor.matmul(zq[:], lhsT=ones_r[:, :128], rhs=binrow_sb[:],
                                 start=False, stop=True)
                nc.scalar.activation(acc[:, g, :], zq[:], AF.Copy,
                                     scale=float(betas[0]))
                z0b = sp.tile([128, HID], BF16, tag="z0b")
                nc.vector.tensor_copy(out=z0b[:], in_=zq[:])
                ztp = psC.tile([128, HID], BF16, tag="C2", space="PSUM")
                for h in range(2):
                    nc.tensor.transpose(out=ztp[:, h * 128:(h + 1) * 128],
                                        in_=z0b[:, h * 128:(h + 1) * 128],
                                        identity=ident_bf[:])
                nc.vector.tensor_copy(
                    out=zview()[:, :, g * 128:(g + 1) * 128],
                    in_=ztp[:].rearrange("p (h n) -> p h n", h=2))

            # ---- layers ----
            for l in range(nlayers):
                w1_sb = wp.tile([128, 2, 128], BF16, tag="w1")
                w2_sb = wp.tile([128, 2, 128], BF16, tag="w2")
                u1_sb = wp.tile([128, 4, 128], BF16, tag="u1")
                u2_sb = wp.tile([128, 2, HID], BF16, tag="u2")
                rows_sb = wp.tile([1, 6 * HID], BF16, tag="rows")
                nc.sync.dma_start(out=w1_sb[:], in_=w1_d[l])
                nc.sync.dma_start(out=w2_sb[:], in_=w2_d[l])
                nc.sync.dma_start(out=u1_sb[:], in_=u1_d[l])
                nc.sync.dma_start(out=u2_sb[:], in_=u2_d[l])
                nc.sync.dma_start(out=rows_sb[:], in_=rows_d[l:l + 1, :])

                def row(i, lo=0, n=HID):
                    return rows_sb[:, i * HID + lo: i * HID + lo + n]

                gb_sb = wp.tile([128, HID], BF16, tag="gb")
                bb_sb = wp.tile([128, HID], BF16, tag="bb")
                for dst_t, ridx in ((gb_sb, 4), (bb_sb, 5)):
                    bc = psC.tile([128, HID], F32, tag="C", space="PSUM")
                    nc.tensor.matmul(bc[:], lhsT=ones_r[:, :128], rhs=row(ridx),
                                     start=True, stop=True)
                    nc.vector.tensor_copy(out=dst_t[:], in_=bc[:])

                # ---- Q phase + AllGather ----
                qown = dp.tile([NPAD, MSG], BF16, tag="qown")
                qfull = dp.tile([NTOT, MSG], BF16, tag="qfull", addr_space="Shared")
                for g in range(G):
                    qp = psC.tile([128, MSG], F32, tag="C", space="PSUM")
                    nc.tensor.matmul(qp[:], lhsT=zT[:, zcols(0, g)],
                                     rhs=w1_sb[:, 0, :], start=True, stop=False)
                    nc.tensor.matmul(qp[:], lhsT=zT[:, zcols(1, g)],
                                     rhs=w1_sb[:, 1, :], start=False, stop=False)
                    nc.tensor.matmul(qp[:], lhsT=ones_r[:, :128],
                                     rhs=row(0, 0, 128), start=False, stop=True)
                    nc.scalar.activation(qkeep[:, g, :], qp[:], AF.Relu)
                    nc.sync.dma_start(out=qown[g * 128:(g + 1) * 128, :],
                                      in_=qkeep[:, g, :])

                nc.gpsimd.collective_compute(
                    "AllGather", OP.bypass,
                    replica_groups=[list(range(CORES))],
                    ins=[qown[:].opt()], outs=[qfull[:].opt()],
                )

                # ---- aggregate + node MLP, software-pipelined by batch ----
                def emit_scatter(bg0, ng):
                    sT = psS.tile([128, 512], F32, tag="S", space="PSUM")
                    for qi in range(ng):
                        g = bg0 + qi
                        qs = slice(qi * 128, (qi + 1) * 128)
                        # self-loop contribution
                        nc.tensor.matmul(sT[:, qs], lhsT=qkeep[:, g, :],
                                         rhs=ident_bf[:], start=True, stop=False)
                        nblk_tot = int(B[g, 0] + B[g, 1])
                        done = 0
                        for h in range(2):
                            nb = int(B[g, h])
                            o = int(seg_off[g, h])
                            gat = gp.tile([128, nbmax, 128], BF16, tag="gat")
                            nc.gpsimd.dma_gather(
                                out_ap=gat[:, :nb, :],
                                in_ap=qfull[h * HALF:(h + 1) * HALF, :],
                                idxs_ap=idx_sb[:, o // 16:(o + nb * 128) // 16],
                                num_idxs=nb * 128,
                                num_idxs_reg=nb * 128,
                                elem_size=MSG,
                                single_packet=False,
                                queue_num=qcall[0] % NQ,
                            )
                            qcall[0] += 1
                            oh = gp.tile([128, nbmax, 128], BF16, tag="oh")
                            nc.vector.tensor_tensor(
                                out=oh[:, :nb, :],
                                in0=iota_bf[:, None, :].to_broadcast([128, nb, 128]),
                                in1=dstv_sb[:, o // 128:o // 128 + nb, None]
                                    .to_broadcast([128, nb, 128]),
                                op=OP.is_equal,
                            )
                            for b in range(nb):
                                done += 1
                                nc.tensor.matmul(
                                    sT[:, qs], lhsT=gat[:, b, :],
                                    rhs=oh[:, b, :],
                                    start=False, stop=(done == nblk_tot),
                                )
                    return sT

                def emit_mlp(bg0, ng, sT):
                    nw = ng * 128
                    snorm = sp.tile([128, 512], BF16, tag="sn")
                    nc.vector.tensor_tensor(
                        out=snorm[:, :nw], in0=sT[:, :nw],
                        in1=cinv_sb[:, bg0 * 128:bg0 * 128 + nw], op=OP.mult)

                    # m (hid-major) = W2^T @ snorm + b2
                    mp = psB.tile([128, 2, 512], F32, tag="B", space="PSUM")
                    for m in range(2):
                        nc.tensor.matmul(mp[:, m, :nw], lhsT=w2_sb[:, m, :],
                                         rhs=snorm[:, :nw], start=True, stop=False)
                        nc.tensor.matmul(mp[:, m, :nw], lhsT=row(1, m * 128, 128),
                                         rhs=ones_r[:, :nw], start=False, stop=True)
                    hT = sp.tile([128, 2, 512], BF16, tag="h", bufs=2)
                    nc.vector.tensor_tensor(
                        out=hT[:, :, :nw],
                        in0=mp[:, :, :nw],
                        in1=zview()[:, :, bg0 * 128:bg0 * 128 + nw],
                        op=OP.add,
                    )

                    # r = relu(U1^T @ h + c1)   (hid-major)
                    rp = psB.tile([128, 2, 512], F32, tag="B", space="PSUM")
                    for m in range(2):
                        nc.tensor.matmul(rp[:, m, :nw], lhsT=u1_sb[:, 0 * 2 + m, :],
                                         rhs=hT[:, 0, :nw], start=True, stop=False)
                        nc.tensor.matmul(rp[:, m, :nw], lhsT=u1_sb[:, 1 * 2 + m, :],
                                         rhs=hT[:, 1, :nw], start=False, stop=False)
                        nc.tensor.matmul(rp[:, m, :nw], lhsT=row(2, m * 128, 128),
                                         rhs=ones_r[:, :nw], start=False, stop=True)
                    rT = sp.tile([128, 2, 512], BF16, tag="rt", bufs=2)
                    nc.scalar.activation(rT[:, :, :nw], rp[:, :, :nw], AF.Relu)

                    s1c = sp.tile([128, BATCH], F32, tag="s1c")
                    s2c = sp.tile([128, BATCH], F32, tag="s2c")
                    o4 = sp.tile([128, BATCH, HID], F32, tag="ob", bufs=2)
                    for qi in range(ng):
                        g = bg0 + qi
                        qs = slice(qi * 128, (qi + 1) * 128)
                        op_ = psC.tile([128, HID], F32, tag="C", space="PSUM")
                        nc.tensor.matmul(op_[:], lhsT=rT[:, 0, qs],
                                         rhs=u2_sb[:, 0, :], start=True, stop=False)
                        nc.tensor.matmul(op_[:], lhsT=rT[:, 1, qs],
                                         rhs=u2_sb[:, 1, :], start=False, stop=False)
                        nc.tensor.matmul(op_[:], lhsT=ones_r[:, :128], rhs=row(3),
                                         start=False, stop=True)
                        junk = sp.tile([128, HID], F32, tag="junk", bufs=1)
                        nc.scalar.activation(o4[:, qi, :], op_[:], AF.Copy,
                                             accum_out=s1c[:, qi:qi + 1])
                        nc.scalar.activation(junk[:], op_[:], AF.Square,
                                             accum_out=s2c[:, qi:qi + 1])

                    mu4 = sp.tile([128, BATCH], F32, tag="mu4")
                    ex24 = sp.tile([128, BATCH], F32, tag="ex24")
                    msq4 = sp.tile([128, BATCH], F32, tag="msq4")
                    var4 = sp.tile([128, BATCH], F32, tag="var4")
                    sd4 = sp.tile([128, BATCH], F32, tag="sd4")
                    rstd4 = sp.tile([128, BATCH], F32, tag="rstd4")
                    nc.vector.tensor_scalar_mul(mu4[:, :ng], s1c[:, :ng], 1.0 / HID)
                    nc.vector.tensor_scalar(out=ex24[:, :ng], in0=s2c[:, :ng],
                                            scalar1=1.0 / HID, scalar2=EPS,
                                            op0=OP.mult, op1=OP.add)
                    nc.scalar.activation(msq4[:, :ng], mu4[:, :ng], AF.Square)
                    nc.vector.tensor_tensor(out=var4[:, :ng], in0=ex24[:, :ng],
                                            in1=msq4[:, :ng], op=OP.subtract)
                    nc.scalar.activation(sd4[:, :ng], var4[:, :ng], AF.Sqrt)
                    nc.vector.reciprocal(rstd4[:, :ng], sd4[:, :ng])

                    for qi in range(ng):
                        g = bg0 + qi
                        zc = sp.tile([128, HID], BF16, tag="zc")
                        nc.vector.tensor_scalar(out=zc[:], in0=o4[:, qi, :],
                                                scalar1=mu4[:, qi:qi + 1],
                                                scalar2=rstd4[:, qi:qi + 1],
                                                op0=OP.subtract, op1=OP.mult)
                        zg = sp.tile([128, HID], BF16, tag="zg")
                        nc.vector.tensor_tensor(out=zg[:], in0=zc[:], in1=gb_sb[:],
                                                op=OP.mult)
                        znm = sp.tile([128, HID], BF16, tag="znm")
                        nc.vector.tensor_tensor(out=znm[:], in0=zg[:], in1=bb_sb[:],
                                                op=OP.add)

                        # acc += beta[l+1] * znm  (scale on ACT, add on DVE)
                        ab = sp.tile([128, HID], F32, tag="ab")
                        nc.scalar.activation(ab[:], znm[:], AF.Copy,
                                             scale=float(betas[l + 1]))
                        nc.vector.tensor_tensor(out=acc[:, g, :], in0=acc[:, g, :],
                                                in1=ab[:], op=OP.add)

                        # z writeback (transpose to feature-major)
                        ztp = psC.tile([128, HID], BF16, tag="C2", space="PSUM")
                        for h in range(2):
                            nc.tensor.transpose(out=ztp[:, h * 128:(h + 1) * 128],
                                                in_=znm[:, h * 128:(h + 1) * 128],
                                                identity=ident_bf[:])
                        nc.vector.tensor_copy(
                            out=zview()[:, :, g * 128:(g + 1) * 128],
                            in_=ztp[:].rearrange("p (h n) -> p h n", h=2))

                prev = None
                for (bg0, ng) in batches:
                    sT_k = emit_scatter(bg0, ng)
                    if prev is not None:
                        emit_mlp(*prev)
                    prev = (bg0, ng, sT_k)
                emit_mlp(*prev)

            # ---- final output ----
            nc.sync.dma_start(
                out=out_d[:].rearrange("(g p) c -> p g c", p=128),
                in_=acc[:])

    nc.compile()
    return nc


def _prep_inputs(inputs, idx_pack, dstv_cols, cinv, perm, nlayers=L):
    x = np.asarray(inputs["x"], np.float32)
    Win = np.asarray(inputs["Win"], np.float32)
    bin_ = np.asarray(inputs["bin_"], np.float32)
    W1 = np.asarray(inputs["W1"], np.float32)
    b1 = np.asarray(inputs["b1"], np.float32)
    W2 = np.asarray(inputs["W2"], np.float32)
    b2 = np.asarray(inputs["b2"], np.float32)
    U1 = np.asarray(inputs["U1"], np.float32)
    c1 = np.asarray(inputs["c1"], np.float32)
    U2 = np.asarray(inputs["U2"], np.float32)
    c2 = np.asarray(inputs["c2"], np.float32)
    ln_g = np.asarray(inputs["ln_g"], np.float32)
    ln_b = np.asarray(inputs["ln_b"], np.float32)
    core_of, loc_of = perm

    w1t = np.ascontiguousarray(
        W1[:nlayers].reshape(nlayers, 2, 128, 128).transpose(0, 2, 1, 3))
    w2t = np.ascontiguousarray(W2[:nlayers].reshape(nlayers, 128, 2, 128))
    u1t = np.ascontiguousarray(
        U1[:nlayers].reshape(nlayers, 2, 128, 2, 128).transpose(0, 2, 1, 3, 4)
        .reshape(nlayers, 128, 4, 128))
    u2t = np.ascontiguousarray(
        U2[:nlayers].reshape(nlayers, 2, 128, HID).transpose(0, 2, 1, 3))
    rows = np.zeros((nlayers, 6, HID), np.float32)
    rows[:, 0, :128] = b1[:nlayers]
    rows[:, 1] = b2[:nlayers]
    rows[:, 2] = c1[:nlayers]
    rows[:, 3] = c2[:nlayers]
    rows[:, 4] = ln_g[:nlayers]
    rows[:, 5] = ln_b[:nlayers]

    shared = {
        "win": Win.astype(BF), "binrow": bin_[None, :].astype(BF),
        "w1t": w1t.astype(BF), "w2t": w2t.astype(BF),
        "u1t": u1t.astype(BF), "u2t": u2t.astype(BF),
        "rows": rows.reshape(nlayers, 6 * HID).astype(BF),
    }
    in_maps = []
    for r in range(CORES):
        sel = core_of == r
        xs = np.zeros((128, NPAD), np.float32)
        xs[:, loc_of[sel]] = x[sel].T
        m = dict(shared)
        m["xT"] = xs.astype(BF)
        m["idxp"] = np.ascontiguousarray(idx_pack[r])
        m["dstv"] = np.ascontiguousarray(dstv_cols[r]).astype(BF)
        m["cinv"] = np.broadcast_to(
            cinv[r].astype(np.float32)[None, :], (128, NPAD)).astype(BF).copy()
        in_maps.append(m)
    return in_maps


def _ensure_ntff_hook():
    """The image's antenv lacks axon_hooks; shim it so trace=True works."""
    import sys as _sys, types as _types
    try:
        from antenv.axon_hooks import get_axon_ntff_profile_hook  # noqa: F401
        return
    except ImportError:
        pass
    mod = _types.ModuleType("antenv.axon_hooks")
    holder = {}
    mod.set_axon_ntff_profile_hook = lambda h: holder.__setitem__("h", h)
    mod.get_axon_ntff_profile_hook = lambda: holder.get("h")
    _sys.modules["antenv.axon_hooks"] = mod
    import antenv
    antenv.axon_hooks = mod
    from trn_agent_boot.trn_boot import _ntff_profile_via_ctypes
    hook = _ntff_profile_via_ctypes("/opt/axon/libaxon_pjrt.so")
    if hook is not None:
        mod.set_axon_ntff_profile_hook(hook)
    import concourse.bass_utils as _bu
    _bu.upload_artifacts = lambda d: d  # no S3 in this sandbox


def kernel(**inputs) -> np.ndarray:
    beta = np.asarray(inputs["beta"], np.float32)
    bmax = beta.max()
    e = np.exp(beta - bmax)
    betas = (e / e.sum()).astype(np.float32)

    import os
    nl = int(os.environ.get("KLAYERS", L))
    B, seg_off, idx_pack, dstv_cols, cinv, totb, perm = _preprocess(
        inputs["edge_index"])
    nc = _build(B, seg_off, totb, betas, nlayers=nl)
    in_maps = _prep_inputs(inputs, idx_pack, dstv_cols, cinv, perm, nlayers=nl)
    try:
        _ensure_ntff_hook()
        res = run_bass_kernel_spmd(nc, in_maps, core_ids=list(range(CORES)),
                                   trace=True)
    except Exception:
        import traceback
        traceback.print_exc()
        res = run_bass_kernel_spmd(nc, in_maps, core_ids=list(range(CORES)))
    globals()["LAST_EXEC_NS"] = res.exec_time_ns or res.mean_exec_time_ns
    globals()["LAST_RESULT"] = res

    core_of, loc_of = perm
    out = np.empty((N, HID), np.float32)
    for r in range(CORES):
        sel = core_of == r
        out[sel] = res.results[r]["out"][loc_of[sel]]
    return out


if __name__ == "__main__":
    rng = np.random.default_rng(0)
    ins = {
        "x": rng.standard_normal((N, IN_CH), dtype=np.float32),
        "edge_index": rng.integers(0, N, size=(2, 800000)).astype(np.int32),
        "Win": rng.standard_normal((IN_CH, HID), dtype=np.float32) * 0.05,
        "bin_": np.zeros(HID, np.float32),
        "W1": rng.standard_normal((L, HID, MSG), dtype=np.float32) * 0.05,
        "b1": np.zeros((L, MSG), np.float32),
        "W2": rng.standard_normal((L, MSG, HID), dtype=np.float32) * 0.05,
        "b2": np.zeros((L, HID), np.float32),
        "U1": rng.standard_normal((L, HID, HID), dtype=np.float32) * 0.05,
        "c1": np.zeros((L, HID), np.float32),
        "U2": rng.standard_normal((L, HID, HID), dtype=np.float32) * 0.05,
        "c2": np.zeros((L, HID), np.float32),
        "ln_g": np.ones((L, HID), np.float32),
        "ln_b": np.zeros((L, HID), np.float32),
        "beta": 0.01 * rng.standard_normal(L + 1).astype(np.float32),
    }
    out = kernel(**ins)
    print(out.shape, out.dtype, np.abs(out).mean())


# revision 29
# speedup vs baseline: 1.0124x; 1.0124x over previous
"""HLMPNN (hierarchical layered MPNN) Bass kernel for 8 TRN2 NeuronCores — v2.

Strategy (graph/data parallel):
  - Nodes assigned to 392 degree-balanced bins of 128 (greedy packing on
    in-degree), 49 bins (groups) per core -> per-(core,group) slot counts
    nearly equal, minimizing block padding and max-over-core waste.
  - Edge MLP decomposed: per-node Q = relu(z@W1+b1) bf16, AllGather Q,
    per-edge dma_gather of Q rows (int16 idx, two half-tables), scatter-add
    via one-hot matmuls into PSUM, then mean-normalize and apply W2.
  - Self-loops are not gathered: added via an identity matmul from local Q.
  - dma_gather calls round-robin across 4 SWDGE queues (Q7 core pairs) for
    parallel descriptor generation (measured ~2.8x).
  - All matmuls bf16 (fp32 is 4 cyc/row on the PE); z-state in bf16;
    LayerNorm statistics fp32.
  - beta-weighted output accumulated in SBUF fp32; one DRAM write at end.
"""
import numpy as np
import ml_dtypes

import concourse.bass as bass
import concourse.bass2jax as _b2j
import concourse.mybir as mybir

_orig_hook = _b2j.neuronx_cc_hook
def _dbg_hook(*a, **k):
    try:
        return _orig_hook(*a, **k)
    except BaseException:
        import traceback
        traceback.print_exc()
        raise
_b2j.neuronx_cc_hook = _dbg_hook
import concourse.tile as tile
from concourse import bacc
from concourse.bass_utils import run_bass_kernel_spmd
from concourse.masks import make_identity

F32 = mybir.dt.float32
BF16 = mybir.dt.bfloat16
I16 = mybir.dt.int16
AF = mybir.ActivationFunctionType
OP = mybir.AluOpType

CORES = 8
N = 50000
IN_CH = 128
HID = 256
MSG = 128
L = 10
EPS = 1e-5
G = 49                       # groups (bins) per core
NPAD = G * 128               # 6272 node slots per core
NTOT = NPAD * CORES          # 50176
HALF = NTOT // 2             # 25088
PADDST = 200.0
NQ = 4                       # SWDGE queues
BATCH = 4                    # groups per MLP batch

BF = ml_dtypes.bfloat16


def _preprocess(edge_index):
    src = np.asarray(edge_index[0], np.int64)
    dst = np.asarray(edge_index[1], np.int64)

    deg = np.bincount(dst, minlength=N).astype(np.int64)   # in-degree, no loop
    counts_node = (deg + 1).astype(np.float64)             # with self-loop

    # --- degree-balanced assignment of nodes to 392 bins of <=128 ---
    import heapq
    NB = CORES * G
    order = np.argsort(-deg, kind="stable")
    heap = [(0, b) for b in range(NB)]
    heapq.heapify(heap)
    bin_of = np.empty(N, np.int32)
    pos_of = np.empty(N, np.int32)
    bin_cnt = np.zeros(NB, np.int32)
    for n in order:
        spill = []
        while True:
            load, b = heapq.heappop(heap)
            if bin_cnt[b] < 128:
                break
            spill.append((load, b))
        bin_of[n] = b
        pos_of[n] = bin_cnt[b]
        bin_cnt[b] += 1
        heapq.heappush(heap, (load + int(deg[n]), b))
        # bins at capacity stay out of the heap permanently
    core_of = (bin_of // G).astype(np.int64)
    grp_of = (bin_of % G).astype(np.int64)
    loc_of = grp_of * 128 + pos_of                  # local row in [0, 6272)
    glob_of = core_of * NPAD + loc_of               # row in qfull

    # --- per-edge slot tables (dst-owner core gathers src rows) ---
    e_core = core_of[dst]
    e_grp = grp_of[dst]
    e_nloc = pos_of[dst].astype(np.int64)
    e_src_glob = glob_of[src]
    e_h = (e_src_glob >= HALF).astype(np.int64)
    e_idx = e_src_glob - e_h * HALF

    order_e = np.lexsort((e_idx, e_h, e_grp, e_core))
    so, sg, sh = e_core[order_e], e_grp[order_e], e_h[order_e]
    si, sn = e_idx[order_e], e_nloc[order_e]
    key = ((so * G) + sg) * 2 + sh
    bounds = np.searchsorted(key, np.arange(CORES * G * 2 + 1))

    cnt = (bounds[1:] - bounds[:-1]).reshape(CORES, G, 2)
    B = np.maximum(1, -(-cnt.max(axis=0) // 128))   # [G, 2] block counts
    seg_off = np.zeros((G, 2), np.int64)
    off = 0
    for g in range(G):
        for h in range(2):
            seg_off[g, h] = off
            off += int(B[g, h]) * 128
    totslots = off
    totb = totslots // 128

    idx16 = np.zeros((CORES, totslots), np.int16)   # pad idx 0 (a real row)
    dstv = np.full((CORES, totslots), PADDST, np.float32)
    for r in range(CORES):
        for g in range(G):
            for h in range(2):
                k = (r * G + g) * 2 + h
                lo, hi = bounds[k], bounds[k + 1]
                o = seg_off[g, h]
                idx16[r, o:o + hi - lo] = si[lo:hi].astype(np.int16)
                dstv[r, o:o + hi - lo] = sn[lo:hi].astype(np.float32)

    idx_pack = np.tile(
        idx16.reshape(CORES, totslots // 16, 16).transpose(0, 2, 1), (1, 8, 1)
    )  # [CORES, 128, totslots//16]
    dstv_cols = dstv.reshape(CORES, totb, 128).transpose(0, 2, 1)  # [C,128,totb]

    cinv = np.ones((CORES, NPAD), np.float64)
    cinv[core_of, loc_of] = 1.0 / counts_node

    perm = (core_of, loc_of)
    return B, seg_off, idx_pack, dstv_cols, cinv, totb, perm


def _build(B, seg_off, totb, betas, nlayers=L):
    nc = bacc.Bacc(None, target_bir_lowering=False, debug=False,
                   num_swdge_queues=NQ)

    xT_d = nc.dram_tensor("xT", [128, NPAD], BF16, kind="ExternalInput")
    win_d = nc.dram_tensor("win", [128, HID], BF16, kind="ExternalInput")
    binrow_d = nc.dram_tensor("binrow", [1, HID], BF16, kind="ExternalInput")
    w1_d = nc.dram_tensor("w1t", [nlayers, 128, 2, 128], BF16, kind="ExternalInput")
    w2_d = nc.dram_tensor("w2t", [nlayers, 128, 2, 128], BF16, kind="ExternalInput")
    u1_d = nc.dram_tensor("u1t", [nlayers, 128, 4, 128], BF16, kind="ExternalInput")
    u2_d = nc.dram_tensor("u2t", [nlayers, 128, 2, HID], BF16, kind="ExternalInput")
    rows_d = nc.dram_tensor("rows", [nlayers, 6 * HID], BF16, kind="ExternalInput")
    idx_d = nc.dram_tensor("idxp", [128, totb * 8], I16, kind="ExternalInput")
    dstv_d = nc.dram_tensor("dstv", [128, totb], BF16, kind="ExternalInput")
    cinv_d = nc.dram_tensor("cinv", [128, NPAD], BF16, kind="ExternalInput")
    out_d = nc.dram_tensor("out", [NPAD, HID], F32, kind="ExternalOutput")

    nbmax = int(B.max())
    qcall = [0]
    dma_sems = [nc.alloc_semaphore(f"gsem{q}") for q in range(NQ)]

    # group batches: [0..3], [4..7], ..., [48]
    batches = []
    g0 = 0
    while g0 < G:
        ng = min(BATCH, G - g0)
        batches.append((g0, ng))
        g0 += ng

    with tile.TileContext(nc) as tc:
        with (
            tc.tile_pool(name="persist", bufs=1) as pp,
            tc.tile_pool(name="wpool", bufs=2) as wp,
            tc.tile_pool(name="stream", bufs=3) as sp,
            tc.tile_pool(name="gath", bufs=8) as gp,
            tc.tile_pool(name="psC", bufs=2, space="PSUM") as psC,
            tc.tile_pool(name="psS", bufs=2, space="PSUM") as psS,
            tc.tile_pool(name="psB", bufs=1, space="PSUM") as psB,
            tc.tile_pool(name="dram", bufs=2, space="DRAM") as dp,
        ):
            # ---- persistent state ----
            zT = pp.tile([128, 2 * NPAD], BF16)      # z feature-major (h, node)
            acc = pp.tile([128, G, HID], F32)        # beta-weighted accumulator
            qkeep = pp.tile([128, G, MSG], BF16)     # own Q node-major per group
            cinv_sb = pp.tile([128, NPAD], BF16)
            dstv_sb = pp.tile([128, totb], BF16)
            idx_sb = pp.tile([128, totb * 8], I16)
            iota_bf = pp.tile([128, 128], BF16)
            ident = pp.tile([128, 128], F32)
            ident_bf = pp.tile([128, 128], BF16)
            ones_r = pp.tile([1, 512], BF16)
            win_sb = pp.tile([128, HID], BF16)
            binrow_sb = pp.tile([1, HID], BF16)

            nc.sync.dma_start(out=cinv_sb[:], in_=cinv_d[:])
            nc.sync.dma_start(out=dstv_sb[:], in_=dstv_d[:])
            nc.sync.dma_start(out=idx_sb[:], in_=idx_d[:])
            nc.sync.dma_start(out=win_sb[:], in_=win_d[:])
            nc.sync.dma_start(out=binrow_sb[:], in_=binrow_d[:])

            iota_i = sp.tile([128, 128], mybir.dt.int32, tag="ioi")
            nc.gpsimd.iota(iota_i[:], pattern=[[1, 128]], base=0, channel_multiplier=0)
            nc.vector.tensor_copy(out=iota_bf[:], in_=iota_i[:])
            make_identity(nc, ident[:])
            nc.vector.tensor_copy(out=ident_bf[:], in_=ident[:])
            nc.vector.memset(ones_r[:], 1.0)
            # warm the gather pool: -1-trimmed pad slots leave stale SBUF
            # content that the one-hot zeros multiply; ensure it is finite
            for _ in range(8):
                gw = gp.tile([128, nbmax, 128], BF16, tag="gat")
                nc.vector.memset(gw[:], 0.0)

            def zview():
                return zT[:].rearrange("p (h n) -> p h n", h=2)

            def zcols(h, g0, ng=1):
                return slice(h * NPAD + g0 * 128, h * NPAD + (g0 + ng) * 128)

            # ---- z0 = x @ Win + bin ----
            for g in range(G):
                xg = sp.tile([128, 128], BF16, tag="xg")
                nc.sync.dma_start(out=xg[:], in_=xT_d[:, g * 128:(g + 1) * 128])
                zq = psC.tile([128, HID], F32, tag="C", space="PSUM")
                nc.tensor.matmul(zq[:], lhsT=xg[:],
                                 rhs=win_sb[:], start=True, stop=False)
                nc.tensor.matmul(zq[:], lhsT=ones_r[:, :128], rhs=binrow_sb[:],
                                 start=False, stop=True)
                nc.scalar.activation(acc[:, g, :], zq[:], AF.Copy,
                                     scale=float(betas[0]))
                z0b = sp.tile([128, HID], BF16, tag="z0b")
                nc.vector.tensor_copy(out=z0b[:], in_=zq[:])
                ztp = psC.tile([128, HID], BF16, tag="C2", space="PSUM")
                for h in range(2):
                    nc.tensor.transpose(out=ztp[:, h * 128:(h + 1) * 128],
                                        in_=z0b[:, h * 128:(h + 1) * 128],
                                        identity=ident_bf[:])
                nc.vector.tensor_copy(
                    out=zview()[:, :, g * 128:(g + 1) * 128],
                    in_=ztp[:].rearrange("p (h n) -> p h n", h=2))

            # ---- layers ----
            for l in range(nlayers):
                w1_sb = wp.tile([128, 2, 128], BF16, tag="w1")
                w2_sb = wp.tile([128, 2, 128], BF16, tag="w2")
                u1_sb = wp.tile([128, 4, 128], BF16, tag="u1")
                u2_sb = wp.tile([128, 2, HID], BF16, tag="u2")
                rows_sb = wp.tile([1, 6 * HID], BF16, tag="rows")
                nc.sync.dma_start(out=w1_sb[:], in_=w1_d[l])
                nc.sync.dma_start(out=w2_sb[:], in_=w2_d[l])
                nc.sync.dma_start(out=u1_sb[:], in_=u1_d[l])
                nc.sync.dma_start(out=u2_sb[:], in_=u2_d[l])
                nc.sync.dma_start(out=rows_sb[:], in_=rows_d[l:l + 1, :])

                def row(i, lo=0, n=HID):
                    return rows_sb[:, i * HID + lo: i * HID + lo + n]

                gb_sb = wp.tile([128, HID], BF16, tag="gb")
                bb_sb = wp.tile([128, HID], BF16, tag="bb")
                for dst_t, ridx in ((gb_sb, 4), (bb_sb, 5)):
                    bc = psC.tile([128, HID], F32, tag="C", space="PSUM")
                    nc.tensor.matmul(bc[:], lhsT=ones_r[:, :128], rhs=row(ridx),
                                     start=True, stop=True)
                    nc.vector.tensor_copy(out=dst_t[:], in_=bc[:])

                # ---- Q phase + AllGather ----
                qown = dp.tile([NPAD, MSG], BF16, tag="qown")
                qfull = dp.tile([NTOT, MSG], BF16, tag="qfull", addr_space="Shared")
                for g in range(G):
                    qp = psC.tile([128, MSG], F32, tag="C", space="PSUM")
                    nc.tensor.matmul(qp[:], lhsT=zT[:, zcols(0, g)],
                                     rhs=w1_sb[:, 0, :], start=True, stop=False)
                    nc.tensor.matmul(qp[:], lhsT=zT[:, zcols(1, g)],
                                     rhs=w1_sb[:, 1, :], start=False, stop=False)
                    nc.tensor.matmul(qp[:], lhsT=ones_r[:, :128],
                                     rhs=row(0, 0, 128), start=False, stop=True)
                    nc.scalar.activation(qkeep[:, g, :], qp[:], AF.Relu)
                    nc.sync.dma_start(out=qown[g * 128:(g + 1) * 128, :],
                                      in_=qkeep[:, g, :])

                nc.gpsimd.collective_compute(
                    "AllGather", OP.bypass,
                    replica_groups=[list(range(CORES))],
                    ins=[qown[:].opt()], outs=[qfull[:].opt()],
                )

                # ---- aggregate + node MLP, software-pipelined by batch ----
                def emit_scatter(bg0, ng):
                    sT = psS.tile([128, 512], F32, tag="S", space="PSUM")
                    for qi in range(ng):
                        g = bg0 + qi
                        qs = slice(qi * 128, (qi + 1) * 128)
                        # self-loop contribution
                        nc.tensor.matmul(sT[:, qs], lhsT=qkeep[:, g, :],
                                         rhs=ident_bf[:], start=True, stop=False)
                        nblk_tot = int(B[g, 0] + B[g, 1])
                        done = 0
                        for h in range(2):
                            nb = int(B[g, h])
                            o = int(seg_off[g, h])
                            gat = gp.tile([128, nbmax, 128], BF16, tag="gat")
                            nc.gpsimd.dma_gather(
                                out_ap=gat[:, :nb, :],
                                in_ap=qfull[h * HALF:(h + 1) * HALF, :],
                                idxs_ap=idx_sb[:, o // 16:(o + nb * 128) // 16],
                                num_idxs=nb * 128,
                                num_idxs_reg=nb * 128,
                                elem_size=MSG,
                                single_packet=False,
                                queue_num=qcall[0] % NQ,
                            )
                            qcall[0] += 1
                            oh = gp.tile([128, nbmax, 128], BF16, tag="oh")
                            nc.vector.tensor_tensor(
                                out=oh[:, :nb, :],
                                in0=iota_bf[:, None, :].to_broadcast([128, nb, 128]),
                                in1=dstv_sb[:, o // 128:o // 128 + nb, None]
                                    .to_broadcast([128, nb, 128]),
                                op=OP.is_equal,
                            )
                            for b in range(nb):
                                done += 1
                                nc.tensor.matmul(
                                    sT[:, qs], lhsT=gat[:, b, :],
                                    rhs=oh[:, b, :],
                                    start=False, stop=(done == nblk_tot),
                                )
                    return sT

                def emit_mlp(bg0, ng, sT):
                    nw = ng * 128
                    snorm = sp.tile([128, 512], BF16, tag="sn")
                    nc.vector.tensor_tensor(
                        out=snorm[:, :nw], in0=sT[:, :nw],
                        in1=cinv_sb[:, bg0 * 128:bg0 * 128 + nw], op=OP.mult)

                    # m (hid-major) = W2^T @ snorm + b2
                    mp = psB.tile([128, 2, 512], F32, tag="B", space="PSUM")
                    for m in range(2):
                        nc.tensor.matmul(mp[:, m, :nw], lhsT=w2_sb[:, m, :],
                                         rhs=snorm[:, :nw], start=True, stop=False)
                        nc.tensor.matmul(mp[:, m, :nw], lhsT=row(1, m * 128, 128),
                                         rhs=ones_r[:, :nw], start=False, stop=True)
                    hT = sp.tile([128, 2, 512], BF16, tag="h", bufs=2)
                    nc.vector.tensor_tensor(
                        out=hT[:, :, :nw],
                        in0=mp[:, :, :nw],
                        in1=zview()[:, :, bg0 * 128:bg0 * 128 + nw],
                        op=OP.add,
                    )

                    # r = relu(U1^T @ h + c1)   (hid-major)
                    rp = psB.tile([128, 2, 512], F32, tag="B", space="PSUM")
                    for m in range(2):
                        nc.tensor.matmul(rp[:, m, :nw], lhsT=u1_sb[:, 0 * 2 + m, :],
                                         rhs=hT[:, 0, :nw], start=True, stop=False)
                        nc.tensor.matmul(rp[:, m, :nw], lhsT=u1_sb[:, 1 * 2 + m, :],
                                         rhs=hT[:, 1, :nw], start=False, stop=False)
                        nc.tensor.matmul(rp[:, m, :nw], lhsT=row(2, m * 128, 128),
                                         rhs=ones_r[:, :nw], start=False, stop=True)
                    rT = sp.tile([128, 2, 512], BF16, tag="rt", bufs=2)
                    nc.scalar.activation(rT[:, :, :nw], rp[:, :, :nw], AF.Relu)

                    s1c = sp.tile([128, BATCH], F32, tag="s1c")
                    s2c = sp.tile([128, BATCH], F32, tag="s2c")
                    o4 = sp.tile([128, BATCH, HID], F32, tag="ob", bufs=2)
                    for qi in range(ng):
                        g = bg0 + qi
                        qs = slice(qi * 128, (qi + 1) * 128)
                        op_ = psC.tile([128, HID], F32, tag="C", space="PSUM")
                        nc.tensor.matmul(op_[:], lhsT=rT[:, 0, qs],
                                         rhs=u2_sb[:, 0, :], start=True, stop=False)
                        nc.tensor.matmul(op_[:], lhsT=rT[:, 1, qs],
                                         rhs=u2_sb[:, 1, :], start=False, stop=False)
                        nc.tensor.matmul(op_[:], lhsT=ones_r[:, :128], rhs=row(3),
                                         start=False, stop=True)
                        junk = sp.tile([128, HID], F32, tag="junk", bufs=1)
                        nc.scalar.activation(o4[:, qi, :], op_[:], AF.Copy,
                                             accum_out=s1c[:, qi:qi + 1])
                        nc.scalar.activation(junk[:], op_[:], AF.Square,
                                             accum_out=s2c[:, qi:qi + 1])

                    mu4 = sp.tile([128, BATCH], F32, tag="mu4")
                    ex24 = sp.tile([128, BATCH], F32, tag="ex24")
                    msq4 = sp.tile([128, BATCH], F32, tag="msq4")
                    var4 = sp.tile([128, BATCH], F32, tag="var4")
                    sd4 = sp.tile([128, BATCH], F32, tag="sd4")
                    rstd4 = sp.tile([128, BATCH], F32, tag="rstd4")
                    nc.vector.tensor_scalar_mul(mu4[:, :ng], s1c[:, :ng], 1.0 / HID)
                    nc.vector.tensor_scalar(out=ex24[:, :ng], in0=s2c[:, :ng],
                                            scalar1=1.0 / HID, scalar2=EPS,
                                            op0=OP.mult, op1=OP.add)
                    nc.scalar.activation(msq4[:, :ng], mu4[:, :ng], AF.Square)
                    nc.vector.tensor_tensor(out=var4[:, :ng], in0=ex24[:, :ng],
                                            in1=msq4[:, :ng], op=OP.subtract)
                    nc.scalar.activation(sd4[:, :ng], var4[:, :ng], AF.Sqrt)
                    nc.vector.reciprocal(rstd4[:, :ng], sd4[:, :ng])

                    for qi in range(ng):
                        g = bg0 + qi
                        zc = sp.tile([128, HID], BF16, tag="zc")
                        nc.vector.tensor_scalar(out=zc[:], in0=o4[:, qi, :],
                                                scalar1=mu4[:, qi:qi + 1],
                                                scalar2=rstd4[:, qi:qi + 1],
                                                op0=OP.subtract, op1=OP.mult)
                        zg = sp.tile([128, HID], BF16, tag="zg")
                        nc.vector.tensor_tensor(out=zg[:], in0=zc[:], in1=gb_sb[:],
                                                op=OP.mult)
                        znm = sp.tile([128, HID], BF16, tag="znm")
                        nc.vector.tensor_tensor(out=znm[:], in0=zg[:], in1=bb_sb[:],
                                                op=OP.add)

                        # acc += beta[l+1] * znm  (scale on ACT, add on DVE)
                        ab = sp.tile([128, HID], F32, tag="ab")
                        nc.scalar.activation(ab[:], znm[:], AF.Copy,
                                             scale=float(betas[l + 1]))
                        nc.vector.tensor_tensor(out=acc[:, g, :], in0=acc[:, g, :],
                                                in1=ab[:], op=OP.add)

                        # z writeback (transpose to feature-major)
                        ztp = psC.tile([128, HID], BF16, tag="C2", space="PSUM")
                        for h in range(2):
                            nc.tensor.transpose(out=ztp[:, h * 128:(h + 1) * 128],
                                                in_=znm[:, h * 128:(h + 1) * 128],
                                                identity=ident_bf[:])
                        nc.vector.tensor_copy(
                            out=zview()[:, :, g * 128:(g + 1) * 128],
                            in_=ztp[:].rearrange("p (h n) -> p h n", h=2))

                prev = None
                for (bg0, ng) in batches:
                    sT_k = emit_scatter(bg0, ng)
                    if prev is not None:
                        emit_mlp(*prev)
                    prev = (bg0, ng, sT_k)
                emit_mlp(*prev)

            # ---- final output ----
            nc.sync.dma_start(
                out=out_d[:].rearrange("(g p) c -> p g c", p=128),
                in_=acc[:])

    nc.compile()
    return nc


def _prep_inputs(inputs, idx_pack, dstv_cols, cinv, perm, nlayers=L):
    x = np.asarray(inputs["x"], np.float32)
    Win = np.asarray(inputs["Win"], np.float32)
    bin_ = np.asarray(inputs["bin_"], np.float32)
    W1 = np.asarray(inputs["W1"], np.float32)
    b1 = np.asarray(inputs["b1"], np.float32)
    W2 = np.asarray(inputs["W2"], np.float32)
    b2 = np.asarray(inputs["b2"], np.float32)
    U1 = np.asarray(inputs["U1"], np.float32)
    c1 = np.asarray(inputs["c1"], np.float32)
    U2 = np.asarray(inputs["U2"], np.float32)
    c2 = np.asarray(inputs["c2"], np.float32)
    ln_g = np.asarray(inputs["ln_g"], np.float32)
    ln_b = np.asarray(inputs["ln_b"], np.float32)
    core_of, loc_of = perm

    w1t = np.ascontiguousarray(
        W1[:nlayers].reshape(nlayers, 2, 128, 128).transpose(0, 2, 1, 3))
    w2t = np.ascontiguousarray(W2[:nlayers].reshape(nlayers, 128, 2, 128))
    u1t = np.ascontiguousarray(
        U1[:nlayers].reshape(nlayers, 2, 128, 2, 128).transpose(0, 2, 1, 3, 4)
        .reshape(nlayers, 128, 4, 128))
    u2t = np.ascontiguousarray(
        U2[:nlayers].reshape(nlayers, 2, 128, HID).transpose(0, 2, 1, 3))
    rows = np.zeros((nlayers, 6, HID), np.float32)
    rows[:, 0, :128] = b1[:nlayers]
    rows[:, 1] = b2[:nlayers]
    rows[:, 2] = c1[:nlayers]
    rows[:, 3] = c2[:nlayers]
    rows[:, 4] = ln_g[:nlayers]
    rows[:, 5] = ln_b[:nlayers]

    shared = {
        "win": Win.astype(BF), "binrow": bin_[None, :].astype(BF),
        "w1t": w1t.astype(BF), "w2t": w2t.astype(BF),
        "u1t": u1t.astype(BF), "u2t": u2t.astype(BF),
        "rows": rows.reshape(nlayers, 6 * HID).astype(BF),
    }
    in_maps = []
    for r in range(CORES):
        sel = core_of == r
        xs = np.zeros((128, NPAD), np.float32)
        xs[:, loc_of[sel]] = x[sel].T
        m = dict(shared)
        m["xT"] = xs.astype(BF)
        m["idxp"] = np.ascontiguousarray(idx_pack[r])
        m["dstv"] = np.ascontiguousarray(dstv_cols[r]).astype(BF)
        m["cinv"] = np.broadcast_to(
            cinv[r].astype(np.float32)[None, :], (128, NPAD)).astype(BF).copy()
        in_maps.append(m)
    return in_maps


def _ensure_ntff_hook():
    """The image's antenv lacks axon_hooks; shim it so trace=True works."""
    import sys as _sys, types as _types
    try:
        from antenv.axon_hooks import get_axon_ntff_profile_hook  # noqa: F401
        return
    except ImportError:
        pass
    mod = _types.ModuleType("antenv.axon_hooks")
    holder = {}
    mod.set_axon_ntff_profile_hook = lambda h: holder.__setitem__("h", h)
    mod.get_axon_ntff_profile_hook = lambda: holder.get("h")
    _sys.modules["antenv.axon_hooks"] = mod
    import antenv
    antenv.axon_hooks = mod
    from trn_agent_boot.trn_boot import _ntff_profile_via_ctypes
    hook = _ntff_profile_via_ctypes("/opt/axon/libaxon_pjrt.so")
    if hook is not None:
        mod.set_axon_ntff_profile_hook(hook)
    import concourse.bass_utils as _bu
    _bu.upload_artifacts = lambda d: d  # no S3 in this sandbox


def kernel(**inputs) -> np.ndarray:
    beta = np.asarray(inputs["beta"], np.float32)
    bmax = beta.max()
    e = np.exp(beta - bmax)
    betas = (e / e.sum()).astype(np.float32)

    import os
    nl = int(os.environ.get("KLAYERS", L))
    B, seg_off, idx_pack, dstv_cols, cinv, totb, perm = _preprocess(
        inputs["edge_index"])
    nc = _build(B, seg_off, totb, betas, nlayers=nl)
    in_maps = _prep_inputs(inputs, idx_pack, dstv_cols, cinv, perm, nlayers=nl)
    try:
        _ensure_ntff_hook()
        res = run_bass_kernel_spmd(nc, in_maps, core_ids=list(range(CORES)),
                                   trace=True)
    except Exception:
        import traceback
        traceback.print_exc()
        res = run_bass_kernel_spmd(nc, in_maps, core_ids=list(range(CORES)))
    globals()["LAST_EXEC_NS"] = res.exec_time_ns or res.mean_exec_time_ns
    globals()["LAST_RESULT"] = res

    core_of, loc_of = perm
    out = np.empty((N, HID), np.float32)
    for r in range(CORES):
        sel = core_of == r
        out[sel] = res.results[r]["out"][loc_of[sel]]
    return out


if __name__ == "__main__":
    rng = np.random.default_rng(0)
    ins = {
        "x": rng.standard_normal((N, IN_CH), dtype=np.float32),
        "edge_index": rng.integers(0, N, size=(2, 800000)).astype(np.int32),
        "Win": rng.standard_normal((IN_CH, HID), dtype=np.float32) * 0.05,
        "bin_": np.zeros(HID, np.float32),
        "W1": rng.standard_normal((L, HID, MSG), dtype=np.float32) * 0.05,
        "b1": np.zeros((L, MSG), np.float32),
        "W2": rng.standard_normal((L, MSG, HID), dtype=np.float32) * 0.05,
        "b2": np.zeros((L, HID), np.float32),
        "U1": rng.standard_normal((L, HID, HID), dtype=np.float32) * 0.05,
        "c1": np.zeros((L, HID), np.float32),
        "U2": rng.standard_normal((L, HID, HID), dtype=np.float32) * 0.05,
        "c2": np.zeros((L, HID), np.float32),
        "ln_g": np.ones((L, HID), np.float32),
        "ln_b": np.zeros((L, HID), np.float32),
        "beta": 0.01 * rng.standard_normal(L + 1).astype(np.float32),
    }
    out = kernel(**ins)
    print(out.shape, out.dtype, np.abs(out).mean())


# revision 30
# speedup vs baseline: 1.1007x; 1.0873x over previous
"""HLMPNN (hierarchical layered MPNN) Bass kernel for 8 TRN2 NeuronCores — v2.

Strategy (graph/data parallel):
  - Nodes assigned to 392 degree-balanced bins of 128 (greedy packing on
    in-degree), 49 bins (groups) per core -> per-(core,group) slot counts
    nearly equal, minimizing block padding and max-over-core waste.
  - Edge MLP decomposed: per-node Q = relu(z@W1+b1) bf16, AllGather Q,
    per-edge dma_gather of Q rows (int16 idx, two half-tables), scatter-add
    via one-hot matmuls into PSUM, then mean-normalize and apply W2.
  - Self-loops are not gathered: added via an identity matmul from local Q.
  - dma_gather calls round-robin across 4 SWDGE queues (Q7 core pairs) for
    parallel descriptor generation (measured ~2.8x).
  - All matmuls bf16 (fp32 is 4 cyc/row on the PE); z-state in bf16;
    LayerNorm statistics fp32.
  - beta-weighted output accumulated in SBUF fp32; one DRAM write at end.
"""
import numpy as np
import ml_dtypes

import concourse.bass as bass
import concourse.bass2jax as _b2j
import concourse.mybir as mybir

_orig_hook = _b2j.neuronx_cc_hook
def _dbg_hook(*a, **k):
    try:
        return _orig_hook(*a, **k)
    except BaseException:
        import traceback
        traceback.print_exc()
        raise
_b2j.neuronx_cc_hook = _dbg_hook
import concourse.tile as tile
from concourse import bacc
from concourse.bass_utils import run_bass_kernel_spmd
from concourse.masks import make_identity

F32 = mybir.dt.float32
BF16 = mybir.dt.bfloat16
I16 = mybir.dt.int16
AF = mybir.ActivationFunctionType
OP = mybir.AluOpType

CORES = 8
N = 50000
IN_CH = 128
HID = 256
MSG = 128
L = 10
EPS = 1e-5
G = 49                       # groups (bins) per core
NPAD = G * 128               # 6272 node slots per core
NTOT = NPAD * CORES          # 50176
HALF = NTOT // 2             # 25088
PADDST = 200.0
NQ = 4                       # SWDGE queues
BATCH = 4                    # groups per MLP batch

BF = ml_dtypes.bfloat16


def _preprocess(edge_index):
    src = np.asarray(edge_index[0], np.int64)
    dst = np.asarray(edge_index[1], np.int64)

    deg = np.bincount(dst, minlength=N).astype(np.int64)   # in-degree, no loop
    counts_node = (deg + 1).astype(np.float64)             # with self-loop

    # --- degree-balanced assignment of nodes to 392 bins of <=128 ---
    import heapq
    NB = CORES * G
    order = np.argsort(-deg, kind="stable")
    heap = [(0, b) for b in range(NB)]
    heapq.heapify(heap)
    bin_of = np.empty(N, np.int32)
    pos_of = np.empty(N, np.int32)
    bin_cnt = np.zeros(NB, np.int32)
    for n in order:
        spill = []
        while True:
            load, b = heapq.heappop(heap)
            if bin_cnt[b] < 128:
                break
            spill.append((load, b))
        bin_of[n] = b
        pos_of[n] = bin_cnt[b]
        bin_cnt[b] += 1
        heapq.heappush(heap, (load + int(deg[n]), b))
        # bins at capacity stay out of the heap permanently
    core_of = (bin_of // G).astype(np.int64)
    grp_of = (bin_of % G).astype(np.int64)
    loc_of = grp_of * 128 + pos_of                  # local row in [0, 6272)
    glob_of = core_of * NPAD + loc_of               # row in qfull

    # --- per-edge slot tables (dst-owner core gathers src rows) ---
    e_core = core_of[dst]
    e_grp = grp_of[dst]
    e_nloc = pos_of[dst].astype(np.int64)
    e_src_glob = glob_of[src]
    e_h = (e_src_glob >= HALF).astype(np.int64)
    e_idx = e_src_glob - e_h * HALF

    order_e = np.lexsort((e_idx, e_h, e_grp, e_core))
    so, sg, sh = e_core[order_e], e_grp[order_e], e_h[order_e]
    si, sn = e_idx[order_e], e_nloc[order_e]
    key = ((so * G) + sg) * 2 + sh
    bounds = np.searchsorted(key, np.arange(CORES * G * 2 + 1))

    cnt = (bounds[1:] - bounds[:-1]).reshape(CORES, G, 2)
    B = np.maximum(1, -(-cnt.max(axis=0) // 128))   # [G, 2] block counts
    seg_off = np.zeros((G, 2), np.int64)
    off = 0
    for g in range(G):
        for h in range(2):
            seg_off[g, h] = off
            off += int(B[g, h]) * 128
    totslots = off
    totb = totslots // 128

    idx16 = np.zeros((CORES, totslots), np.int16)   # pad idx 0 (a real row)
    dstv = np.full((CORES, totslots), PADDST, np.float32)
    for r in range(CORES):
        for g in range(G):
            for h in range(2):
                k = (r * G + g) * 2 + h
                lo, hi = bounds[k], bounds[k + 1]
                o = seg_off[g, h]
                idx16[r, o:o + hi - lo] = si[lo:hi].astype(np.int16)
                dstv[r, o:o + hi - lo] = sn[lo:hi].astype(np.float32)

    idx_pack = np.tile(
        idx16.reshape(CORES, totslots // 16, 16).transpose(0, 2, 1), (1, 8, 1)
    )  # [CORES, 128, totslots//16]
    dstv_cols = dstv.reshape(CORES, totb, 128).transpose(0, 2, 1)  # [C,128,totb]

    cinv = np.ones((CORES, NPAD), np.float64)
    cinv[core_of, loc_of] = 1.0 / counts_node

    perm = (core_of, loc_of)
    return B, seg_off, idx_pack, dstv_cols, cinv, totb, perm


def _build(B, seg_off, totb, betas, nlayers=L):
    nc = bacc.Bacc(None, target_bir_lowering=False, debug=False,
                   num_swdge_queues=NQ)

    xT_d = nc.dram_tensor("xT", [128, NPAD], BF16, kind="ExternalInput")
    win_d = nc.dram_tensor("win", [128, HID], BF16, kind="ExternalInput")
    binrow_d = nc.dram_tensor("binrow", [1, HID], BF16, kind="ExternalInput")
    w1_d = nc.dram_tensor("w1t", [nlayers, 128, 2, 128], BF16, kind="ExternalInput")
    w2_d = nc.dram_tensor("w2t", [nlayers, 128, 2, 128], BF16, kind="ExternalInput")
    u1_d = nc.dram_tensor("u1t", [nlayers, 128, 4, 128], BF16, kind="ExternalInput")
    u2_d = nc.dram_tensor("u2t", [nlayers, 128, 2, HID], BF16, kind="ExternalInput")
    rows_d = nc.dram_tensor("rows", [nlayers, 6 * HID], BF16, kind="ExternalInput")
    idx_d = nc.dram_tensor("idxp", [128, totb * 8], I16, kind="ExternalInput")
    dstv_d = nc.dram_tensor("dstv", [128, totb], BF16, kind="ExternalInput")
    cinv_d = nc.dram_tensor("cinv", [128, NPAD], BF16, kind="ExternalInput")
    out_d = nc.dram_tensor("out", [NPAD, HID], F32, kind="ExternalOutput")

    nbmax = int(B.max())
    qcall = [0]
    dma_sems = [nc.alloc_semaphore(f"gsem{q}") for q in range(NQ)]

    # group batches: [0..3], [4..7], ..., [48]
    batches = []
    g0 = 0
    while g0 < G:
        ng = min(BATCH, G - g0)
        batches.append((g0, ng))
        g0 += ng

    with tile.TileContext(nc) as tc:
        with (
            tc.tile_pool(name="persist", bufs=1) as pp,
            tc.tile_pool(name="wpool", bufs=2) as wp,
            tc.tile_pool(name="stream", bufs=3) as sp,
            tc.tile_pool(name="gath", bufs=8) as gp,
            tc.tile_pool(name="psC", bufs=2, space="PSUM") as psC,
            tc.tile_pool(name="psS", bufs=2, space="PSUM") as psS,
            tc.tile_pool(name="psB", bufs=1, space="PSUM") as psB,
            tc.tile_pool(name="dram", bufs=2, space="DRAM") as dp,
        ):
            # ---- persistent state ----
            zT = pp.tile([128, 2 * NPAD], BF16)      # z feature-major (h, node)
            acc = pp.tile([128, G, HID], F32)        # beta-weighted accumulator
            qkeep = pp.tile([128, G, MSG], BF16)     # own Q node-major per group
            cinv_sb = pp.tile([128, NPAD], BF16)
            dstv_sb = pp.tile([128, totb], BF16)
            idx_sb = pp.tile([128, totb * 8], I16)
            iota_bf = pp.tile([128, 128], BF16)
            ident = pp.tile([128, 128], F32)
            ident_bf = pp.tile([128, 128], BF16)
            ones_r = pp.tile([1, 512], BF16)
            win_sb = pp.tile([128, HID], BF16)
            binrow_sb = pp.tile([1, HID], BF16)

            nc.sync.dma_start(out=cinv_sb[:], in_=cinv_d[:])
            nc.sync.dma_start(out=dstv_sb[:], in_=dstv_d[:])
            nc.sync.dma_start(out=idx_sb[:], in_=idx_d[:])
            nc.sync.dma_start(out=win_sb[:], in_=win_d[:])
            nc.sync.dma_start(out=binrow_sb[:], in_=binrow_d[:])

            iota_i = sp.tile([128, 128], mybir.dt.int32, tag="ioi")
            nc.gpsimd.iota(iota_i[:], pattern=[[1, 128]], base=0, channel_multiplier=0)
            nc.vector.tensor_copy(out=iota_bf[:], in_=iota_i[:])
            make_identity(nc, ident[:])
            nc.vector.tensor_copy(out=ident_bf[:], in_=ident[:])
            nc.vector.memset(ones_r[:], 1.0)
            # warm the gather pool: -1-trimmed pad slots leave stale SBUF
            # content that the one-hot zeros multiply; ensure it is finite
            for _ in range(8):
                gw = gp.tile([128, nbmax, 128], BF16, tag="gat")
                nc.vector.memset(gw[:], 0.0)

            def zview():
                return zT[:].rearrange("p (h n) -> p h n", h=2)

            def zcols(h, g0, ng=1):
                return slice(h * NPAD + g0 * 128, h * NPAD + (g0 + ng) * 128)

            # ---- z0 = x @ Win + bin ----
            for g in range(G):
                xg = sp.tile([128, 128], BF16, tag="xg")
                nc.sync.dma_start(out=xg[:], in_=xT_d[:, g * 128:(g + 1) * 128])
                zq = psC.tile([128, HID], F32, tag="C", space="PSUM")
                nc.tensor.matmul(zq[:], lhsT=xg[:],
                                 rhs=win_sb[:], start=True, stop=False)
                nc.tensor.matmul(zq[:], lhsT=ones_r[:, :128], rhs=binrow_sb[:],
                                 start=False, stop=True)
                nc.scalar.activation(acc[:, g, :], zq[:], AF.Copy,
                                     scale=float(betas[0]))
                z0b = sp.tile([128, HID], BF16, tag="z0b")
                nc.vector.tensor_copy(out=z0b[:], in_=zq[:])
                ztp = psC.tile([128, HID], BF16, tag="C2", space="PSUM")
                for h in range(2):
                    nc.tensor.transpose(out=ztp[:, h * 128:(h + 1) * 128],
                                        in_=z0b[:, h * 128:(h + 1) * 128],
                                        identity=ident_bf[:])
                nc.vector.tensor_copy(
                    out=zview()[:, :, g * 128:(g + 1) * 128],
                    in_=ztp[:].rearrange("p (h n) -> p h n", h=2))

            # ---- layers ----
            for l in range(nlayers):
                w1_sb = wp.tile([128, 2, 128], BF16, tag="w1")
                w2_sb = wp.tile([128, 2, 128], BF16, tag="w2")
                u1_sb = wp.tile([128, 4, 128], BF16, tag="u1")
                u2_sb = wp.tile([128, 2, HID], BF16, tag="u2")
                rows_sb = wp.tile([1, 6 * HID], BF16, tag="rows")
                nc.sync.dma_start(out=w1_sb[:], in_=w1_d[l])
                nc.sync.dma_start(out=w2_sb[:], in_=w2_d[l])
                nc.sync.dma_start(out=u1_sb[:], in_=u1_d[l])
                nc.sync.dma_start(out=u2_sb[:], in_=u2_d[l])
                nc.sync.dma_start(out=rows_sb[:], in_=rows_d[l:l + 1, :])

                def row(i, lo=0, n=HID):
                    return rows_sb[:, i * HID + lo: i * HID + lo + n]

                gb_sb = wp.tile([128, HID], BF16, tag="gb")
                bb_sb = wp.tile([128, HID], BF16, tag="bb")
                for dst_t, ridx in ((gb_sb, 4), (bb_sb, 5)):
                    bc = psC.tile([128, HID], F32, tag="C", space="PSUM")
                    nc.tensor.matmul(bc[:], lhsT=ones_r[:, :128], rhs=row(ridx),
                                     start=True, stop=True)
                    nc.vector.tensor_copy(out=dst_t[:], in_=bc[:])

                # ---- Q phase + AllGather ----
                qown = dp.tile([NPAD, MSG], BF16, tag="qown")
                qfull = dp.tile([NTOT, MSG], BF16, tag="qfull", addr_space="Shared")
                for g in range(G):
                    qp = psC.tile([128, MSG], F32, tag="C", space="PSUM")
                    nc.tensor.matmul(qp[:], lhsT=zT[:, zcols(0, g)],
                                     rhs=w1_sb[:, 0, :], start=True, stop=False)
                    nc.tensor.matmul(qp[:], lhsT=zT[:, zcols(1, g)],
                                     rhs=w1_sb[:, 1, :], start=False, stop=False)
                    nc.tensor.matmul(qp[:], lhsT=ones_r[:, :128],
                                     rhs=row(0, 0, 128), start=False, stop=True)
                    nc.scalar.activation(qkeep[:, g, :], qp[:], AF.Relu)
                    nc.sync.dma_start(out=qown[g * 128:(g + 1) * 128, :],
                                      in_=qkeep[:, g, :])

                nc.gpsimd.collective_compute(
                    "AllGather", OP.bypass,
                    replica_groups=[list(range(CORES))],
                    ins=[qown[:].opt()], outs=[qfull[:].opt()],
                )

                # ---- aggregate + node MLP, software-pipelined by batch ----
                def emit_scatter(bg0, ng):
                    sT = psS.tile([128, 512], F32, tag="S", space="PSUM")
                    for qi in range(ng):
                        g = bg0 + qi
                        qs = slice(qi * 128, (qi + 1) * 128)
                        # self-loop contribution
                        nc.tensor.matmul(sT[:, qs], lhsT=qkeep[:, g, :],
                                         rhs=ident_bf[:], start=True, stop=False)
                        nblk_tot = int(B[g, 0] + B[g, 1])
                        done = 0
                        for h in range(2):
                            nb = int(B[g, h])
                            o = int(seg_off[g, h])
                            gat = gp.tile([128, nbmax, 128], BF16, tag="gat")
                            # split each segment's gather across two SWDGE
                            # queues: finer desc-gen quanta fill the 4 Q7
                            # pairs better (same tile, disjoint block ranges)
                            nb1 = (nb + 1) // 2
                            for (b0, bn) in ((0, nb1), (nb1, nb - nb1)):
                                if bn <= 0:
                                    continue
                                oo = o + b0 * 128
                                nc.gpsimd.dma_gather(
                                    out_ap=gat[:, b0:b0 + bn, :],
                                    in_ap=qfull[h * HALF:(h + 1) * HALF, :],
                                    idxs_ap=idx_sb[:, oo // 16:
                                                   (oo + bn * 128) // 16],
                                    num_idxs=bn * 128,
                                    num_idxs_reg=bn * 128,
                                    elem_size=MSG,
                                    single_packet=False,
                                    queue_num=qcall[0] % NQ,
                                )
                                qcall[0] += 1
                            oh = gp.tile([128, nbmax, 128], BF16, tag="oh")
                            nc.vector.tensor_tensor(
                                out=oh[:, :nb, :],
                                in0=iota_bf[:, None, :].to_broadcast([128, nb, 128]),
                                in1=dstv_sb[:, o // 128:o // 128 + nb, None]
                                    .to_broadcast([128, nb, 128]),
                                op=OP.is_equal,
                            )
                            for b in range(nb):
                                done += 1
                                nc.tensor.matmul(
                                    sT[:, qs], lhsT=gat[:, b, :],
                                    rhs=oh[:, b, :],
                                    start=False, stop=(done == nblk_tot),
                                )
                    return sT

                def emit_mlp(bg0, ng, sT):
                    nw = ng * 128
                    snorm = sp.tile([128, 512], BF16, tag="sn")
                    nc.vector.tensor_tensor(
                        out=snorm[:, :nw], in0=sT[:, :nw],
                        in1=cinv_sb[:, bg0 * 128:bg0 * 128 + nw], op=OP.mult)

                    # m (hid-major) = W2^T @ snorm + b2
                    mp = psB.tile([128, 2, 512], F32, tag="B", space="PSUM")
                    for m in range(2):
                        nc.tensor.matmul(mp[:, m, :nw], lhsT=w2_sb[:, m, :],
                                         rhs=snorm[:, :nw], start=True, stop=False)
                        nc.tensor.matmul(mp[:, m, :nw], lhsT=row(1, m * 128, 128),
                                         rhs=ones_r[:, :nw], start=False, stop=True)
                    hT = sp.tile([128, 2, 512], BF16, tag="h", bufs=2)
                    nc.vector.tensor_tensor(
                        out=hT[:, :, :nw],
                        in0=mp[:, :, :nw],
                        in1=zview()[:, :, bg0 * 128:bg0 * 128 + nw],
                        op=OP.add,
                    )

                    # r = relu(U1^T @ h + c1)   (hid-major)
                    rp = psB.tile([128, 2, 512], F32, tag="B", space="PSUM")
                    for m in range(2):
                        nc.tensor.matmul(rp[:, m, :nw], lhsT=u1_sb[:, 0 * 2 + m, :],
                                         rhs=hT[:, 0, :nw], start=True, stop=False)
                        nc.tensor.matmul(rp[:, m, :nw], lhsT=u1_sb[:, 1 * 2 + m, :],
                                         rhs=hT[:, 1, :nw], start=False, stop=False)
                        nc.tensor.matmul(rp[:, m, :nw], lhsT=row(2, m * 128, 128),
                                         rhs=ones_r[:, :nw], start=False, stop=True)
                    rT = sp.tile([128, 2, 512], BF16, tag="rt", bufs=2)
                    nc.scalar.activation(rT[:, :, :nw], rp[:, :, :nw], AF.Relu)

                    s1c = sp.tile([128, BATCH], F32, tag="s1c")
                    s2c = sp.tile([128, BATCH], F32, tag="s2c")
                    o4 = sp.tile([128, BATCH, HID], F32, tag="ob", bufs=2)
                    for qi in range(ng):
                        g = bg0 + qi
                        qs = slice(qi * 128, (qi + 1) * 128)
                        op_ = psC.tile([128, HID], F32, tag="C", space="PSUM")
                        nc.tensor.matmul(op_[:], lhsT=rT[:, 0, qs],
                                         rhs=u2_sb[:, 0, :], start=True, stop=False)
                        nc.tensor.matmul(op_[:], lhsT=rT[:, 1, qs],
                                         rhs=u2_sb[:, 1, :], start=False, stop=False)
                        nc.tensor.matmul(op_[:], lhsT=ones_r[:, :128], rhs=row(3),
                                         start=False, stop=True)
                        junk = sp.tile([128, HID], F32, tag="junk", bufs=1)
                        nc.scalar.activation(o4[:, qi, :], op_[:], AF.Copy,
                                             accum_out=s1c[:, qi:qi + 1])
                        nc.scalar.activation(junk[:], op_[:], AF.Square,
                                             accum_out=s2c[:, qi:qi + 1])

                    mu4 = sp.tile([128, BATCH], F32, tag="mu4")
                    ex24 = sp.tile([128, BATCH], F32, tag="ex24")
                    msq4 = sp.tile([128, BATCH], F32, tag="msq4")
                    var4 = sp.tile([128, BATCH], F32, tag="var4")
                    sd4 = sp.tile([128, BATCH], F32, tag="sd4")
                    rstd4 = sp.tile([128, BATCH], F32, tag="rstd4")
                    nc.vector.tensor_scalar_mul(mu4[:, :ng], s1c[:, :ng], 1.0 / HID)
                    nc.vector.tensor_scalar(out=ex24[:, :ng], in0=s2c[:, :ng],
                                            scalar1=1.0 / HID, scalar2=EPS,
                                            op0=OP.mult, op1=OP.add)
                    nc.scalar.activation(msq4[:, :ng], mu4[:, :ng], AF.Square)
                    nc.vector.tensor_tensor(out=var4[:, :ng], in0=ex24[:, :ng],
                                            in1=msq4[:, :ng], op=OP.subtract)
                    nc.scalar.activation(sd4[:, :ng], var4[:, :ng], AF.Sqrt)
                    nc.vector.reciprocal(rstd4[:, :ng], sd4[:, :ng])

                    for qi in range(ng):
                        g = bg0 + qi
                        zc = sp.tile([128, HID], BF16, tag="zc")
                        nc.vector.tensor_scalar(out=zc[:], in0=o4[:, qi, :],
                                                scalar1=mu4[:, qi:qi + 1],
                                                scalar2=rstd4[:, qi:qi + 1],
                                                op0=OP.subtract, op1=OP.mult)
                        zg = sp.tile([128, HID], BF16, tag="zg")
                        nc.vector.tensor_tensor(out=zg[:], in0=zc[:], in1=gb_sb[:],
                                                op=OP.mult)
                        znm = sp.tile([128, HID], BF16, tag="znm")
                        nc.vector.tensor_tensor(out=znm[:], in0=zg[:], in1=bb_sb[:],
                                                op=OP.add)

                        # acc += beta[l+1] * znm  (scale on ACT, add on DVE)
                        ab = sp.tile([128, HID], F32, tag="ab")
                        nc.scalar.activation(ab[:], znm[:], AF.Copy,
                                             scale=float(betas[l + 1]))
                        nc.vector.tensor_tensor(out=acc[:, g, :], in0=acc[:, g, :],
                                                in1=ab[:], op=OP.add)

                        # z writeback (transpose to feature-major)
                        ztp = psC.tile([128, HID], BF16, tag="C2", space="PSUM")
                        for h in range(2):
                            nc.tensor.transpose(out=ztp[:, h * 128:(h + 1) * 128],
                                                in_=znm[:, h * 128:(h + 1) * 128],
                                                identity=ident_bf[:])
                        nc.vector.tensor_copy(
                            out=zview()[:, :, g * 128:(g + 1) * 128],
                            in_=ztp[:].rearrange("p (h n) -> p h n", h=2))

                prev = None
                for (bg0, ng) in batches:
                    sT_k = emit_scatter(bg0, ng)
                    if prev is not None:
                        emit_mlp(*prev)
                    prev = (bg0, ng, sT_k)
                emit_mlp(*prev)

            # ---- final output ----
            nc.sync.dma_start(
                out=out_d[:].rearrange("(g p) c -> p g c", p=128),
                in_=acc[:])

    nc.compile()
    return nc


def _prep_inputs(inputs, idx_pack, dstv_cols, cinv, perm, nlayers=L):
    x = np.asarray(inputs["x"], np.float32)
    Win = np.asarray(inputs["Win"], np.float32)
    bin_ = np.asarray(inputs["bin_"], np.float32)
    W1 = np.asarray(inputs["W1"], np.float32)
    b1 = np.asarray(inputs["b1"], np.float32)
    W2 = np.asarray(inputs["W2"], np.float32)
    b2 = np.asarray(inputs["b2"], np.float32)
    U1 = np.asarray(inputs["U1"], np.float32)
    c1 = np.asarray(inputs["c1"], np.float32)
    U2 = np.asarray(inputs["U2"], np.float32)
    c2 = np.asarray(inputs["c2"], np.float32)
    ln_g = np.asarray(inputs["ln_g"], np.float32)
    ln_b = np.asarray(inputs["ln_b"], np.float32)
    core_of, loc_of = perm

    w1t = np.ascontiguousarray(
        W1[:nlayers].reshape(nlayers, 2, 128, 128).transpose(0, 2, 1, 3))
    w2t = np.ascontiguousarray(W2[:nlayers].reshape(nlayers, 128, 2, 128))
    u1t = np.ascontiguousarray(
        U1[:nlayers].reshape(nlayers, 2, 128, 2, 128).transpose(0, 2, 1, 3, 4)
        .reshape(nlayers, 128, 4, 128))
    u2t = np.ascontiguousarray(
        U2[:nlayers].reshape(nlayers, 2, 128, HID).transpose(0, 2, 1, 3))
    rows = np.zeros((nlayers, 6, HID), np.float32)
    rows[:, 0, :128] = b1[:nlayers]
    rows[:, 1] = b2[:nlayers]
    rows[:, 2] = c1[:nlayers]
    rows[:, 3] = c2[:nlayers]
    rows[:, 4] = ln_g[:nlayers]
    rows[:, 5] = ln_b[:nlayers]

    shared = {
        "win": Win.astype(BF), "binrow": bin_[None, :].astype(BF),
        "w1t": w1t.astype(BF), "w2t": w2t.astype(BF),
        "u1t": u1t.astype(BF), "u2t": u2t.astype(BF),
        "rows": rows.reshape(nlayers, 6 * HID).astype(BF),
    }
    in_maps = []
    for r in range(CORES):
        sel = core_of == r
        xs = np.zeros((128, NPAD), np.float32)
        xs[:, loc_of[sel]] = x[sel].T
        m = dict(shared)
        m["xT"] = xs.astype(BF)
        m["idxp"] = np.ascontiguousarray(idx_pack[r])
        m["dstv"] = np.ascontiguousarray(dstv_cols[r]).astype(BF)
        m["cinv"] = np.broadcast_to(
            cinv[r].astype(np.float32)[None, :], (128, NPAD)).astype(BF).copy()
        in_maps.append(m)
    return in_maps


def _ensure_ntff_hook():
    """The image's antenv lacks axon_hooks; shim it so trace=True works."""
    import sys as _sys, types as _types
    try:
        from antenv.axon_hooks import get_axon_ntff_profile_hook  # noqa: F401
        return
    except ImportError:
        pass
    mod = _types.ModuleType("antenv.axon_hooks")
    holder = {}
    mod.set_axon_ntff_profile_hook = lambda h: holder.__setitem__("h", h)
    mod.get_axon_ntff_profile_hook = lambda: holder.get("h")
    _sys.modules["antenv.axon_hooks"] = mod
    import antenv
    antenv.axon_hooks = mod
    from trn_agent_boot.trn_boot import _ntff_profile_via_ctypes
    hook = _ntff_profile_via_ctypes("/opt/axon/libaxon_pjrt.so")
    if hook is not None:
        mod.set_axon_ntff_profile_hook(hook)
    import concourse.bass_utils as _bu
    _bu.upload_artifacts = lambda d: d  # no S3 in this sandbox


def kernel(**inputs) -> np.ndarray:
    beta = np.asarray(inputs["beta"], np.float32)
    bmax = beta.max()
    e = np.exp(beta - bmax)
    betas = (e / e.sum()).astype(np.float32)

    import os
    nl = int(os.environ.get("KLAYERS", L))
    B, seg_off, idx_pack, dstv_cols, cinv, totb, perm = _preprocess(
        inputs["edge_index"])
    nc = _build(B, seg_off, totb, betas, nlayers=nl)
    in_maps = _prep_inputs(inputs, idx_pack, dstv_cols, cinv, perm, nlayers=nl)
    try:
        _ensure_ntff_hook()
        res = run_bass_kernel_spmd(nc, in_maps, core_ids=list(range(CORES)),
                                   trace=True)
    except Exception:
        import traceback
        traceback.print_exc()
        res = run_bass_kernel_spmd(nc, in_maps, core_ids=list(range(CORES)))
    globals()["LAST_EXEC_NS"] = res.exec_time_ns or res.mean_exec_time_ns
    globals()["LAST_RESULT"] = res

    core_of, loc_of = perm
    out = np.empty((N, HID), np.float32)
    for r in range(CORES):
        sel = core_of == r
        out[sel] = res.results[r]["out"][loc_of[sel]]
    return out


if __name__ == "__main__":
    rng = np.random.default_rng(0)
    ins = {
        "x": rng.standard_normal((N, IN_CH), dtype=np.float32),
        "edge_index": rng.integers(0, N, size=(2, 800000)).astype(np.int32),
        "Win": rng.standard_normal((IN_CH, HID), dtype=np.float32) * 0.05,
        "bin_": np.zeros(HID, np.float32),
        "W1": rng.standard_normal((L, HID, MSG), dtype=np.float32) * 0.05,
        "b1": np.zeros((L, MSG), np.float32),
        "W2": rng.standard_normal((L, MSG, HID), dtype=np.float32) * 0.05,
        "b2": np.zeros((L, HID), np.float32),
        "U1": rng.standard_normal((L, HID, HID), dtype=np.float32) * 0.05,
        "c1": np.zeros((L, HID), np.float32),
        "U2": rng.standard_normal((L, HID, HID), dtype=np.float32) * 0.05,
        "c2": np.zeros((L, HID), np.float32),
        "ln_g": np.ones((L, HID), np.float32),
        "ln_b": np.zeros((L, HID), np.float32),
        "beta": 0.01 * rng.standard_normal(L + 1).astype(np.float32),
    }
    out = kernel(**ins)
    print(out.shape, out.dtype, np.abs(out).mean())


# revision 31
# speedup vs baseline: 1.1801x; 1.0721x over previous
"""HLMPNN (hierarchical layered MPNN) Bass kernel for 8 TRN2 NeuronCores — v2.

Strategy (graph/data parallel):
  - Nodes assigned to 392 degree-balanced bins of 128 (greedy packing on
    in-degree), 49 bins (groups) per core -> per-(core,group) slot counts
    nearly equal, minimizing block padding and max-over-core waste.
  - Edge MLP decomposed: per-node Q = relu(z@W1+b1) bf16, AllGather Q,
    per-edge dma_gather of Q rows (int16 idx, two half-tables), scatter-add
    via one-hot matmuls into PSUM, then mean-normalize and apply W2.
  - Self-loops are not gathered: added via an identity matmul from local Q.
  - dma_gather calls round-robin across 4 SWDGE queues (Q7 core pairs) for
    parallel descriptor generation (measured ~2.8x).
  - All matmuls bf16 (fp32 is 4 cyc/row on the PE); z-state in bf16;
    LayerNorm statistics fp32.
  - beta-weighted output accumulated in SBUF fp32; one DRAM write at end.
"""
import numpy as np
import ml_dtypes

import concourse.bass as bass
import concourse.bass2jax as _b2j
import concourse.mybir as mybir

_orig_hook = _b2j.neuronx_cc_hook
def _dbg_hook(*a, **k):
    try:
        return _orig_hook(*a, **k)
    except BaseException:
        import traceback
        traceback.print_exc()
        raise
_b2j.neuronx_cc_hook = _dbg_hook
import concourse.tile as tile
from concourse import bacc
from concourse.bass_utils import run_bass_kernel_spmd
from concourse.masks import make_identity

F32 = mybir.dt.float32
BF16 = mybir.dt.bfloat16
I16 = mybir.dt.int16
AF = mybir.ActivationFunctionType
OP = mybir.AluOpType

CORES = 8
N = 50000
IN_CH = 128
HID = 256
MSG = 128
L = 10
EPS = 1e-5
G = 49                       # groups (bins) per core
NPAD = G * 128               # 6272 node slots per core
NTOT = NPAD * CORES          # 50176
HALF = NTOT // 2             # 25088
PADDST = 200.0
NQ = 4                       # SWDGE queues
BATCH = 4                    # groups per MLP batch

BF = ml_dtypes.bfloat16


def _preprocess(edge_index):
    src = np.asarray(edge_index[0], np.int64)
    dst = np.asarray(edge_index[1], np.int64)

    deg = np.bincount(dst, minlength=N).astype(np.int64)   # in-degree, no loop
    counts_node = (deg + 1).astype(np.float64)             # with self-loop

    # --- degree-balanced assignment of nodes to 392 bins of <=128 ---
    import heapq
    NB = CORES * G
    order = np.argsort(-deg, kind="stable")
    heap = [(0, b) for b in range(NB)]
    heapq.heapify(heap)
    bin_of = np.empty(N, np.int32)
    pos_of = np.empty(N, np.int32)
    bin_cnt = np.zeros(NB, np.int32)
    for n in order:
        spill = []
        while True:
            load, b = heapq.heappop(heap)
            if bin_cnt[b] < 128:
                break
            spill.append((load, b))
        bin_of[n] = b
        pos_of[n] = bin_cnt[b]
        bin_cnt[b] += 1
        heapq.heappush(heap, (load + int(deg[n]), b))
        # bins at capacity stay out of the heap permanently
    core_of = (bin_of // G).astype(np.int64)
    grp_of = (bin_of % G).astype(np.int64)
    loc_of = grp_of * 128 + pos_of                  # local row in [0, 6272)
    glob_of = core_of * NPAD + loc_of               # row in qfull

    # --- per-edge slot tables (dst-owner core gathers src rows) ---
    e_core = core_of[dst]
    e_grp = grp_of[dst]
    e_nloc = pos_of[dst].astype(np.int64)
    e_src_glob = glob_of[src]
    e_h = (e_src_glob >= HALF).astype(np.int64)
    e_idx = e_src_glob - e_h * HALF

    order_e = np.lexsort((e_idx, e_h, e_grp, e_core))
    so, sg, sh = e_core[order_e], e_grp[order_e], e_h[order_e]
    si, sn = e_idx[order_e], e_nloc[order_e]
    key = ((so * G) + sg) * 2 + sh
    bounds = np.searchsorted(key, np.arange(CORES * G * 2 + 1))

    cnt = (bounds[1:] - bounds[:-1]).reshape(CORES, G, 2)
    B = np.maximum(1, -(-cnt.max(axis=0) // 128))   # [G, 2] block counts
    seg_off = np.zeros((G, 2), np.int64)
    off = 0
    for g in range(G):
        for h in range(2):
            seg_off[g, h] = off
            off += int(B[g, h]) * 128
    totslots = off
    totb = totslots // 128

    idx16 = np.zeros((CORES, totslots), np.int16)   # pad idx 0 (a real row)
    dstv = np.full((CORES, totslots), PADDST, np.float32)
    for r in range(CORES):
        for g in range(G):
            for h in range(2):
                k = (r * G + g) * 2 + h
                lo, hi = bounds[k], bounds[k + 1]
                o = seg_off[g, h]
                idx16[r, o:o + hi - lo] = si[lo:hi].astype(np.int16)
                dstv[r, o:o + hi - lo] = sn[lo:hi].astype(np.float32)

    idx_pack = np.tile(
        idx16.reshape(CORES, totslots // 16, 16).transpose(0, 2, 1), (1, 8, 1)
    )  # [CORES, 128, totslots//16]
    dstv_cols = dstv.reshape(CORES, totb, 128).transpose(0, 2, 1)  # [C,128,totb]

    cinv = np.ones((CORES, NPAD), np.float64)
    cinv[core_of, loc_of] = 1.0 / counts_node

    perm = (core_of, loc_of)
    return B, seg_off, idx_pack, dstv_cols, cinv, totb, perm


def _build(B, seg_off, totb, betas, nlayers=L):
    nc = bacc.Bacc(None, target_bir_lowering=False, debug=False,
                   num_swdge_queues=NQ)

    xT_d = nc.dram_tensor("xT", [128, NPAD], BF16, kind="ExternalInput")
    win_d = nc.dram_tensor("win", [128, HID], BF16, kind="ExternalInput")
    binrow_d = nc.dram_tensor("binrow", [1, HID], BF16, kind="ExternalInput")
    w1_d = nc.dram_tensor("w1t", [nlayers, 128, 2, 128], BF16, kind="ExternalInput")
    w2_d = nc.dram_tensor("w2t", [nlayers, 128, 2, 128], BF16, kind="ExternalInput")
    u1_d = nc.dram_tensor("u1t", [nlayers, 128, 4, 128], BF16, kind="ExternalInput")
    u2_d = nc.dram_tensor("u2t", [nlayers, 128, 2, HID], BF16, kind="ExternalInput")
    rows_d = nc.dram_tensor("rows", [nlayers, 6 * HID], BF16, kind="ExternalInput")
    idx_d = nc.dram_tensor("idxp", [128, totb * 8], I16, kind="ExternalInput")
    dstv_d = nc.dram_tensor("dstv", [128, totb], BF16, kind="ExternalInput")
    cinv_d = nc.dram_tensor("cinv", [128, NPAD], BF16, kind="ExternalInput")
    out_d = nc.dram_tensor("out", [NPAD, HID], F32, kind="ExternalOutput")

    nbmax = int(B.max())
    qcall = [0]
    dma_sems = [nc.alloc_semaphore(f"gsem{q}") for q in range(NQ)]

    # group batches: [0..3], [4..7], ..., [48]
    batches = []
    g0 = 0
    while g0 < G:
        ng = min(BATCH, G - g0)
        batches.append((g0, ng))
        g0 += ng

    with tile.TileContext(nc) as tc:
        with (
            tc.tile_pool(name="persist", bufs=1) as pp,
            tc.tile_pool(name="wpool", bufs=2) as wp,
            tc.tile_pool(name="stream", bufs=3) as sp,
            tc.tile_pool(name="gath", bufs=8) as gp,
            tc.tile_pool(name="psC", bufs=2, space="PSUM") as psC,
            tc.tile_pool(name="psS", bufs=2, space="PSUM") as psS,
            tc.tile_pool(name="psB", bufs=1, space="PSUM") as psB,
            tc.tile_pool(name="dram", bufs=2, space="DRAM") as dp,
        ):
            # ---- persistent state ----
            zT = pp.tile([128, 2 * NPAD], BF16)      # z feature-major (h, node)
            acc = pp.tile([128, G, HID], F32)        # beta-weighted accumulator
            qkeep = pp.tile([128, G, MSG], BF16)     # own Q node-major per group
            cinv_sb = pp.tile([128, NPAD], BF16)
            dstv_sb = pp.tile([128, totb], BF16)
            idx_sb = pp.tile([128, totb * 8], I16)
            iota_bf = pp.tile([128, 128], BF16)
            ident = pp.tile([128, 128], F32)
            ident_bf = pp.tile([128, 128], BF16)
            ones_r = pp.tile([1, 512], BF16)
            win_sb = pp.tile([128, HID], BF16)
            binrow_sb = pp.tile([1, HID], BF16)

            nc.sync.dma_start(out=cinv_sb[:], in_=cinv_d[:])
            nc.sync.dma_start(out=dstv_sb[:], in_=dstv_d[:])
            nc.sync.dma_start(out=idx_sb[:], in_=idx_d[:])
            nc.sync.dma_start(out=win_sb[:], in_=win_d[:])
            nc.sync.dma_start(out=binrow_sb[:], in_=binrow_d[:])

            iota_i = sp.tile([128, 128], mybir.dt.int32, tag="ioi")
            nc.gpsimd.iota(iota_i[:], pattern=[[1, 128]], base=0, channel_multiplier=0)
            nc.vector.tensor_copy(out=iota_bf[:], in_=iota_i[:])
            make_identity(nc, ident[:])
            nc.vector.tensor_copy(out=ident_bf[:], in_=ident[:])
            nc.vector.memset(ones_r[:], 1.0)
            # warm the gather pool: -1-trimmed pad slots leave stale SBUF
            # content that the one-hot zeros multiply; ensure it is finite
            for _ in range(8):
                gw = gp.tile([128, nbmax, 128], BF16, tag="gat")
                nc.vector.memset(gw[:], 0.0)

            def zview():
                return zT[:].rearrange("p (h n) -> p h n", h=2)

            def zcols(h, g0, ng=1):
                return slice(h * NPAD + g0 * 128, h * NPAD + (g0 + ng) * 128)

            # ---- z0 = x @ Win + bin ----
            for g in range(G):
                xg = sp.tile([128, 128], BF16, tag="xg")
                nc.sync.dma_start(out=xg[:], in_=xT_d[:, g * 128:(g + 1) * 128])
                zq = psC.tile([128, HID], F32, tag="C", space="PSUM")
                nc.tensor.matmul(zq[:], lhsT=xg[:],
                                 rhs=win_sb[:], start=True, stop=False)
                nc.tensor.matmul(zq[:], lhsT=ones_r[:, :128], rhs=binrow_sb[:],
                                 start=False, stop=True)
                nc.scalar.activation(acc[:, g, :], zq[:], AF.Copy,
                                     scale=float(betas[0]))
                z0b = sp.tile([128, HID], BF16, tag="z0b")
                nc.vector.tensor_copy(out=z0b[:], in_=zq[:])
                ztp = psC.tile([128, HID], BF16, tag="C2", space="PSUM")
                for h in range(2):
                    nc.tensor.transpose(out=ztp[:, h * 128:(h + 1) * 128],
                                        in_=z0b[:, h * 128:(h + 1) * 128],
                                        identity=ident_bf[:])
                nc.vector.tensor_copy(
                    out=zview()[:, :, g * 128:(g + 1) * 128],
                    in_=ztp[:].rearrange("p (h n) -> p h n", h=2))

            # ---- layers ----
            for l in range(nlayers):
                w1_sb = wp.tile([128, 2, 128], BF16, tag="w1")
                w2_sb = wp.tile([128, 2, 128], BF16, tag="w2")
                u1_sb = wp.tile([128, 4, 128], BF16, tag="u1")
                u2_sb = wp.tile([128, 2, HID], BF16, tag="u2")
                rows_sb = wp.tile([1, 6 * HID], BF16, tag="rows")
                nc.sync.dma_start(out=w1_sb[:], in_=w1_d[l])
                nc.sync.dma_start(out=w2_sb[:], in_=w2_d[l])
                nc.sync.dma_start(out=u1_sb[:], in_=u1_d[l])
                nc.sync.dma_start(out=u2_sb[:], in_=u2_d[l])
                nc.sync.dma_start(out=rows_sb[:], in_=rows_d[l:l + 1, :])

                def row(i, lo=0, n=HID):
                    return rows_sb[:, i * HID + lo: i * HID + lo + n]

                gb_sb = wp.tile([128, HID], BF16, tag="gb")
                bb_sb = wp.tile([128, HID], BF16, tag="bb")
                for dst_t, ridx in ((gb_sb, 4), (bb_sb, 5)):
                    bc = psC.tile([128, HID], F32, tag="C", space="PSUM")
                    nc.tensor.matmul(bc[:], lhsT=ones_r[:, :128], rhs=row(ridx),
                                     start=True, stop=True)
                    nc.vector.tensor_copy(out=dst_t[:], in_=bc[:])

                # ---- Q phase + AllGather ----
                qown = dp.tile([NPAD, MSG], BF16, tag="qown")
                qfull = dp.tile([NTOT, MSG], BF16, tag="qfull", addr_space="Shared")
                for g in range(G):
                    qp = psC.tile([128, MSG], F32, tag="C", space="PSUM")
                    nc.tensor.matmul(qp[:], lhsT=zT[:, zcols(0, g)],
                                     rhs=w1_sb[:, 0, :], start=True, stop=False)
                    nc.tensor.matmul(qp[:], lhsT=zT[:, zcols(1, g)],
                                     rhs=w1_sb[:, 1, :], start=False, stop=False)
                    nc.tensor.matmul(qp[:], lhsT=ones_r[:, :128],
                                     rhs=row(0, 0, 128), start=False, stop=True)
                    nc.scalar.activation(qkeep[:, g, :], qp[:], AF.Relu)
                    nc.sync.dma_start(out=qown[g * 128:(g + 1) * 128, :],
                                      in_=qkeep[:, g, :])

                nc.gpsimd.collective_compute(
                    "AllGather", OP.bypass,
                    replica_groups=[list(range(CORES))],
                    ins=[qown[:].opt()], outs=[qfull[:].opt()],
                )

                # ---- aggregate + node MLP, software-pipelined by batch ----
                def emit_scatter(bg0, ng):
                    sT = psS.tile([128, 512], F32, tag="S", space="PSUM")
                    for qi in range(ng):
                        g = bg0 + qi
                        qs = slice(qi * 128, (qi + 1) * 128)
                        # self-loop contribution
                        nc.tensor.matmul(sT[:, qs], lhsT=qkeep[:, g, :],
                                         rhs=ident_bf[:], start=True, stop=False)
                        nblk_tot = int(B[g, 0] + B[g, 1])
                        done = 0
                        for h in range(2):
                            nb = int(B[g, h])
                            o = int(seg_off[g, h])
                            gat = gp.tile([128, nbmax, 128], BF16, tag="gat")
                            # split each segment's gather across three SWDGE
                            # queues: finer desc-gen quanta fill the 4 Q7
                            # pairs better (same tile, disjoint block ranges)
                            parts = []
                            b0 = 0
                            for i in range(3):
                                bn = nb // 3 + (1 if i < nb % 3 else 0)
                                parts.append((b0, bn))
                                b0 += bn
                            for (b0, bn) in parts:
                                if bn <= 0:
                                    continue
                                oo = o + b0 * 128
                                nc.gpsimd.dma_gather(
                                    out_ap=gat[:, b0:b0 + bn, :],
                                    in_ap=qfull[h * HALF:(h + 1) * HALF, :],
                                    idxs_ap=idx_sb[:, oo // 16:
                                                   (oo + bn * 128) // 16],
                                    num_idxs=bn * 128,
                                    num_idxs_reg=bn * 128,
                                    elem_size=MSG,
                                    single_packet=False,
                                    queue_num=qcall[0] % NQ,
                                )
                                qcall[0] += 1
                            oh = gp.tile([128, nbmax, 128], BF16, tag="oh")
                            nc.vector.tensor_tensor(
                                out=oh[:, :nb, :],
                                in0=iota_bf[:, None, :].to_broadcast([128, nb, 128]),
                                in1=dstv_sb[:, o // 128:o // 128 + nb, None]
                                    .to_broadcast([128, nb, 128]),
                                op=OP.is_equal,
                            )
                            for b in range(nb):
                                done += 1
                                nc.tensor.matmul(
                                    sT[:, qs], lhsT=gat[:, b, :],
                                    rhs=oh[:, b, :],
                                    start=False, stop=(done == nblk_tot),
                                )
                    return sT

                def emit_mlp(bg0, ng, sT):
                    nw = ng * 128
                    snorm = sp.tile([128, 512], BF16, tag="sn")
                    nc.vector.tensor_tensor(
                        out=snorm[:, :nw], in0=sT[:, :nw],
                        in1=cinv_sb[:, bg0 * 128:bg0 * 128 + nw], op=OP.mult)

                    # m (hid-major) = W2^T @ snorm + b2
                    mp = psB.tile([128, 2, 512], F32, tag="B", space="PSUM")
                    for m in range(2):
                        nc.tensor.matmul(mp[:, m, :nw], lhsT=w2_sb[:, m, :],
                                         rhs=snorm[:, :nw], start=True, stop=False)
                        nc.tensor.matmul(mp[:, m, :nw], lhsT=row(1, m * 128, 128),
                                         rhs=ones_r[:, :nw], start=False, stop=True)
                    hT = sp.tile([128, 2, 512], BF16, tag="h", bufs=2)
                    nc.vector.tensor_tensor(
                        out=hT[:, :, :nw],
                        in0=mp[:, :, :nw],
                        in1=zview()[:, :, bg0 * 128:bg0 * 128 + nw],
                        op=OP.add,
                    )

                    # r = relu(U1^T @ h + c1)   (hid-major)
                    rp = psB.tile([128, 2, 512], F32, tag="B", space="PSUM")
                    for m in range(2):
                        nc.tensor.matmul(rp[:, m, :nw], lhsT=u1_sb[:, 0 * 2 + m, :],
                                         rhs=hT[:, 0, :nw], start=True, stop=False)
                        nc.tensor.matmul(rp[:, m, :nw], lhsT=u1_sb[:, 1 * 2 + m, :],
                                         rhs=hT[:, 1, :nw], start=False, stop=False)
                        nc.tensor.matmul(rp[:, m, :nw], lhsT=row(2, m * 128, 128),
                                         rhs=ones_r[:, :nw], start=False, stop=True)
                    rT = sp.tile([128, 2, 512], BF16, tag="rt", bufs=2)
                    nc.scalar.activation(rT[:, :, :nw], rp[:, :, :nw], AF.Relu)

                    s1c = sp.tile([128, BATCH], F32, tag="s1c")
                    s2c = sp.tile([128, BATCH], F32, tag="s2c")
                    o4 = sp.tile([128, BATCH, HID], F32, tag="ob", bufs=2)
                    for qi in range(ng):
                        g = bg0 + qi
                        qs = slice(qi * 128, (qi + 1) * 128)
                        op_ = psC.tile([128, HID], F32, tag="C", space="PSUM")
                        nc.tensor.matmul(op_[:], lhsT=rT[:, 0, qs],
                                         rhs=u2_sb[:, 0, :], start=True, stop=False)
                        nc.tensor.matmul(op_[:], lhsT=rT[:, 1, qs],
                                         rhs=u2_sb[:, 1, :], start=False, stop=False)
                        nc.tensor.matmul(op_[:], lhsT=ones_r[:, :128], rhs=row(3),
                                         start=False, stop=True)
                        junk = sp.tile([128, HID], F32, tag="junk", bufs=1)
                        nc.scalar.activation(o4[:, qi, :], op_[:], AF.Copy,
                                             accum_out=s1c[:, qi:qi + 1])
                        nc.scalar.activation(junk[:], op_[:], AF.Square,
                                             accum_out=s2c[:, qi:qi + 1])

                    mu4 = sp.tile([128, BATCH], F32, tag="mu4")
                    ex24 = sp.tile([128, BATCH], F32, tag="ex24")
                    msq4 = sp.tile([128, BATCH], F32, tag="msq4")
                    var4 = sp.tile([128, BATCH], F32, tag="var4")
                    sd4 = sp.tile([128, BATCH], F32, tag="sd4")
                    rstd4 = sp.tile([128, BATCH], F32, tag="rstd4")
                    nc.vector.tensor_scalar_mul(mu4[:, :ng], s1c[:, :ng], 1.0 / HID)
                    nc.vector.tensor_scalar(out=ex24[:, :ng], in0=s2c[:, :ng],
                                            scalar1=1.0 / HID, scalar2=EPS,
                                            op0=OP.mult, op1=OP.add)
                    nc.scalar.activation(msq4[:, :ng], mu4[:, :ng], AF.Square)
                    nc.vector.tensor_tensor(out=var4[:, :ng], in0=ex24[:, :ng],
                                            in1=msq4[:, :ng], op=OP.subtract)
                    nc.scalar.activation(sd4[:, :ng], var4[:, :ng], AF.Sqrt)
                    nc.vector.reciprocal(rstd4[:, :ng], sd4[:, :ng])

                    for qi in range(ng):
                        g = bg0 + qi
                        zc = sp.tile([128, HID], BF16, tag="zc")
                        nc.vector.tensor_scalar(out=zc[:], in0=o4[:, qi, :],
                                                scalar1=mu4[:, qi:qi + 1],
                                                scalar2=rstd4[:, qi:qi + 1],
                                                op0=OP.subtract, op1=OP.mult)
                        zg = sp.tile([128, HID], BF16, tag="zg")
                        nc.vector.tensor_tensor(out=zg[:], in0=zc[:], in1=gb_sb[:],
                                                op=OP.mult)
                        znm = sp.tile([128, HID], BF16, tag="znm")
                        nc.vector.tensor_tensor(out=znm[:], in0=zg[:], in1=bb_sb[:],
                                                op=OP.add)

                        # acc += beta[l+1] * znm  (scale on ACT, add on DVE)
                        ab = sp.tile([128, HID], F32, tag="ab")
                        nc.scalar.activation(ab[:], znm[:], AF.Copy,
                                             scale=float(betas[l + 1]))
                        nc.vector.tensor_tensor(out=acc[:, g, :], in0=acc[:, g, :],
                                                in1=ab[:], op=OP.add)

                        # z writeback (transpose to feature-major)
                        ztp = psC.tile([128, HID], BF16, tag="C2", space="PSUM")
                        for h in range(2):
                            nc.tensor.transpose(out=ztp[:, h * 128:(h + 1) * 128],
                                                in_=znm[:, h * 128:(h + 1) * 128],
                                                identity=ident_bf[:])
                        nc.vector.tensor_copy(
                            out=zview()[:, :, g * 128:(g + 1) * 128],
                            in_=ztp[:].rearrange("p (h n) -> p h n", h=2))

                prev = None
                for (bg0, ng) in batches:
                    sT_k = emit_scatter(bg0, ng)
                    if prev is not None:
                        emit_mlp(*prev)
                    prev = (bg0, ng, sT_k)
                emit_mlp(*prev)

            # ---- final output ----
            nc.sync.dma_start(
                out=out_d[:].rearrange("(g p) c -> p g c", p=128),
                in_=acc[:])

    nc.compile()
    return nc


def _prep_inputs(inputs, idx_pack, dstv_cols, cinv, perm, nlayers=L):
    x = np.asarray(inputs["x"], np.float32)
    Win = np.asarray(inputs["Win"], np.float32)
    bin_ = np.asarray(inputs["bin_"], np.float32)
    W1 = np.asarray(inputs["W1"], np.float32)
    b1 = np.asarray(inputs["b1"], np.float32)
    W2 = np.asarray(inputs["W2"], np.float32)
    b2 = np.asarray(inputs["b2"], np.float32)
    U1 = np.asarray(inputs["U1"], np.float32)
    c1 = np.asarray(inputs["c1"], np.float32)
    U2 = np.asarray(inputs["U2"], np.float32)
    c2 = np.asarray(inputs["c2"], np.float32)
    ln_g = np.asarray(inputs["ln_g"], np.float32)
    ln_b = np.asarray(inputs["ln_b"], np.float32)
    core_of, loc_of = perm

    w1t = np.ascontiguousarray(
        W1[:nlayers].reshape(nlayers, 2, 128, 128).transpose(0, 2, 1, 3))
    w2t = np.ascontiguousarray(W2[:nlayers].reshape(nlayers, 128, 2, 128))
    u1t = np.ascontiguousarray(
        U1[:nlayers].reshape(nlayers, 2, 128, 2, 128).transpose(0, 2, 1, 3, 4)
        .reshape(nlayers, 128, 4, 128))
    u2t = np.ascontiguousarray(
        U2[:nlayers].reshape(nlayers, 2, 128, HID).transpose(0, 2, 1, 3))
    rows = np.zeros((nlayers, 6, HID), np.float32)
    rows[:, 0, :128] = b1[:nlayers]
    rows[:, 1] = b2[:nlayers]
    rows[:, 2] = c1[:nlayers]
    rows[:, 3] = c2[:nlayers]
    rows[:, 4] = ln_g[:nlayers]
    rows[:, 5] = ln_b[:nlayers]

    shared = {
        "win": Win.astype(BF), "binrow": bin_[None, :].astype(BF),
        "w1t": w1t.astype(BF), "w2t": w2t.astype(BF),
        "u1t": u1t.astype(BF), "u2t": u2t.astype(BF),
        "rows": rows.reshape(nlayers, 6 * HID).astype(BF),
    }
    in_maps = []
    for r in range(CORES):
        sel = core_of == r
        xs = np.zeros((128, NPAD), np.float32)
        xs[:, loc_of[sel]] = x[sel].T
        m = dict(shared)
        m["xT"] = xs.astype(BF)
        m["idxp"] = np.ascontiguousarray(idx_pack[r])
        m["dstv"] = np.ascontiguousarray(dstv_cols[r]).astype(BF)
        m["cinv"] = np.broadcast_to(
            cinv[r].astype(np.float32)[None, :], (128, NPAD)).astype(BF).copy()
        in_maps.append(m)
    return in_maps


def _ensure_ntff_hook():
    """The image's antenv lacks axon_hooks; shim it so trace=True works."""
    import sys as _sys, types as _types
    try:
        from antenv.axon_hooks import get_axon_ntff_profile_hook  # noqa: F401
        return
    except ImportError:
        pass
    mod = _types.ModuleType("antenv.axon_hooks")
    holder = {}
    mod.set_axon_ntff_profile_hook = lambda h: holder.__setitem__("h", h)
    mod.get_axon_ntff_profile_hook = lambda: holder.get("h")
    _sys.modules["antenv.axon_hooks"] = mod
    import antenv
    antenv.axon_hooks = mod
    from trn_agent_boot.trn_boot import _ntff_profile_via_ctypes
    hook = _ntff_profile_via_ctypes("/opt/axon/libaxon_pjrt.so")
    if hook is not None:
        mod.set_axon_ntff_profile_hook(hook)
    import concourse.bass_utils as _bu
    _bu.upload_artifacts = lambda d: d  # no S3 in this sandbox


def kernel(**inputs) -> np.ndarray:
    beta = np.asarray(inputs["beta"], np.float32)
    bmax = beta.max()
    e = np.exp(beta - bmax)
    betas = (e / e.sum()).astype(np.float32)

    import os
    nl = int(os.environ.get("KLAYERS", L))
    B, seg_off, idx_pack, dstv_cols, cinv, totb, perm = _preprocess(
        inputs["edge_index"])
    nc = _build(B, seg_off, totb, betas, nlayers=nl)
    in_maps = _prep_inputs(inputs, idx_pack, dstv_cols, cinv, perm, nlayers=nl)
    try:
        _ensure_ntff_hook()
        res = run_bass_kernel_spmd(nc, in_maps, core_ids=list(range(CORES)),
                                   trace=True)
    except Exception:
        import traceback
        traceback.print_exc()
        res = run_bass_kernel_spmd(nc, in_maps, core_ids=list(range(CORES)))
    globals()["LAST_EXEC_NS"] = res.exec_time_ns or res.mean_exec_time_ns
    globals()["LAST_RESULT"] = res

    core_of, loc_of = perm
    out = np.empty((N, HID), np.float32)
    for r in range(CORES):
        sel = core_of == r
        out[sel] = res.results[r]["out"][loc_of[sel]]
    return out


if __name__ == "__main__":
    rng = np.random.default_rng(0)
    ins = {
        "x": rng.standard_normal((N, IN_CH), dtype=np.float32),
        "edge_index": rng.integers(0, N, size=(2, 800000)).astype(np.int32),
        "Win": rng.standard_normal((IN_CH, HID), dtype=np.float32) * 0.05,
        "bin_": np.zeros(HID, np.float32),
        "W1": rng.standard_normal((L, HID, MSG), dtype=np.float32) * 0.05,
        "b1": np.zeros((L, MSG), np.float32),
        "W2": rng.standard_normal((L, MSG, HID), dtype=np.float32) * 0.05,
        "b2": np.zeros((L, HID), np.float32),
        "U1": rng.standard_normal((L, HID, HID), dtype=np.float32) * 0.05,
        "c1": np.zeros((L, HID), np.float32),
        "U2": rng.standard_normal((L, HID, HID), dtype=np.float32) * 0.05,
        "c2": np.zeros((L, HID), np.float32),
        "ln_g": np.ones((L, HID), np.float32),
        "ln_b": np.zeros((L, HID), np.float32),
        "beta": 0.01 * rng.standard_normal(L + 1).astype(np.float32),
    }
    out = kernel(**ins)
    print(out.shape, out.dtype, np.abs(out).mean())
